# revision 43
# baseline (speedup 1.0000x reference)
"""Trainium2 Bass kernel for ArccosHessianCalculator (int8-packed output).

Math: for each batch element b (z1, z2 are [B, D] with D = 128):
  a = 1/|z1|, bb = 1/|z2|, c = cos = <z1u, z2u>
  Each Hessian block H_k is rank-2 plus a diagonal term:
      H_k(b) = z1 * r0_k(b)^T + z2 * r1_k(b)^T + diag
  with the normalization / cosine factors folded into r0/r1:
      k=0 (H11): r0 = -3c*a^4*z1 + a^3 b*z2          r1 = a^3 b*z1
      k=1 (H12): r0 = a^3 b*z1                        r1 = -c*a^2 b^2*z1 + a b^3*z2
      k=2 (H22): r0 = a b^3*z2                        r1 = a b^3*z1 - 3c*b^4*z2

Device strategy (per core, batch shard of 512):
  - TensorE: one K=2 matmul per element, lhsT = [z1(b); z2(b)] ([2,128] fp16),
    rhs = [r0 | r1] blocks ([2,384] fp16) -> PSUM f32 [128, 384]. Elements
    alternate PE quadrant rows ({0,1} / {32,33}) so LDWEIGHTS for element
    e+1 overlaps the stream of element e.
  - INT8 output: a guaranteed per-element bound on |H_rank2| (triangle
    inequality with |c|<=1 and AM-GM, ~16 [128,1] ops) gives s = 126/bound.
    s is folded ENTIRELY into the R coefficients, so the z fp16 casts have
    NO stats dependency (lhsT gathers fire at t=0) and PSUM holds s*H; the
    PSUM->SBUF copy is a plain int8 cast. The rank-2 part lands in DRAM
    packed as out_hw [128 i, B_SH*384 (b,k,j)] int8 -- 4x less HBM traffic
    than f32.
  - Final diagonals are computed in closed form per group ([128 b, 3*128 i])
    into a small f32 side tensor and spliced on the HOST in f32 (they
    dominate the Hessian's magnitude, so they carry no quantization error).
  - PSUM: 2-bank super-tiles x4 buffers; each drained by ONE strided copy
    (2 elements) split 5:6 between DVE and ACT (weighted by their measured
    copy rates; PSUM reads are f32 at ~1 elem/cycle/partition).
  - Stats: group 0-1 chains on DVE (short ramp); groups 2-3 and ALL diag
    (phase2) work on GPSIMD, which is otherwise idle, so recurring stats
    never steal copy throughput. Free-axis reduces / reciprocal stay on DVE
    (GPSIMD cannot do them), sqrt on ACT. All z tiles prefetch at t=0.
  - Batch rows are loaded interleaved (even elements -> partitions 0..63,
    odd -> 64..127) so gathers are plain partition slices; z and R live in
    combined tiles (zh [128,256], Rall [128,768]) so each gather half is a
    single 2-partition DMA (4 dispatches per group instead of 8).
  - Host: reshape/transpose view + per-element dequant multiply + f32
    diagonal stride-trick splice.
"""

import numpy as np
from contextlib import ExitStack

import concourse.bass as bass
import concourse.tile as tile
from concourse import bacc, mybir
from concourse.bass_utils import run_bass_kernel_spmd

N_CORES = 8
B_FULL = 4096
D = 128
B_SH = B_FULL // N_CORES  # 512 batch elements per core
P = 128                   # SBUF partitions
KD = 3 * D                # 384: three H blocks side by side
G = 32                    # elements per chunk
GROUPS = B_SH // P        # 4 stats groups of 128 elements
NCH = P // G              # 4 chunks per group
QB = 2                    # elements per PSUM super-tile (2 banks)
OC = 320                  # output cols per element (256 main + 64 packed corners)
HF = P // 2               # 64 elements per interleaved half

f32 = mybir.dt.float32
f16 = mybir.dt.float16
i8 = mybir.dt.int8

# interleaved element order within a group: partition p holds group element
# 2p (p < 64) or 2(p-64)+1 (p >= 64)
ELEM_OF_PART = np.concatenate([np.arange(0, P, 2), np.arange(1, P, 2)])
PART_OF_ELEM = np.argsort(ELEM_OF_PART)

# copy-engine schedule: 6 ACT : 5 DVE interleaved (ACT is ~1.18x faster at
# PSUM->SBUF f32 copies: (172+FD)/1.2GHz vs (120+FD)/0.96GHz).
# The first 18 super-tiles run 2:1 ACT-heavy: DVE is still draining the
# group-1 stats chain during the ramp-out.
COPY_PAT = [1, 0, 1, 0, 1, 0, 1, 0, 1, 0, 1]  # 1 = ACT


def _copy_engine(q):
    if q < 18:
        return 1 if q % 3 != 2 else 0
    return COPY_PAT[q % len(COPY_PAT)]


class _Pools:
    pass


def _make_pools(ctx, tc):
    p = _Pools()
    p.const = ctx.enter_context(tc.tile_pool(name="const", bufs=1))
    p.zg = ctx.enter_context(tc.tile_pool(name="zg", bufs=1))
    p.zh = ctx.enter_context(tc.tile_pool(name="zh", bufs=4))
    p.work = ctx.enter_context(tc.tile_pool(name="work", bufs=2))
    p.nrm = ctx.enter_context(tc.tile_pool(name="nrm", bufs=1))
    p.stat = ctx.enter_context(tc.tile_pool(name="stat", bufs=3))
    p.rall = ctx.enter_context(tc.tile_pool(name="rall", bufs=2))
    p.dpool = ctx.enter_context(tc.tile_pool(name="dpool", bufs=2))
    p.zi = ctx.enter_context(tc.tile_pool(name="zi", bufs=2))
    p.ri = ctx.enter_context(tc.tile_pool(name="ri", bufs=2))
    p.stage = ctx.enter_context(tc.tile_pool(name="stage", bufs=5))
    p.mmp = ctx.enter_context(tc.tile_pool(name="mmp", bufs=4, space="PSUM"))
    return p


def _make_consts(p, nc):
    """[128,1] constant tiles for GPSIMD (Pool has no scalar immediates)."""
    for tag, val in (("c3", 3.0), ("c075", 0.75), ("c126", 126.0),
                     ("cm1", -1.0), ("cm3", -3.0)):
        t = p.const.tile([P, 1], f32, tag=tag, name=tag)
        nc.vector.memset(t[:], val)
        setattr(p, tag, t)


def _load_all_z(p, nc, z1, z2):
    """Prefetch every group's z tiles at t=0 (tiny: 512B/partition each)."""
    zs = {}
    for grp in range(GROUPS):
        b0 = grp * P
        ldma = nc.sync if grp == 0 else nc.gpsimd
        # interleaved row order: partition p <- batch row 2p / 2(p-64)+1
        z1g = p.zg.tile([P, D], f32, tag=f"z1g{grp}", name=f"z1g_{grp}")
        ldma.dma_start(z1g[:], z1[b0:b0 + P, :].rearrange(
            "(f two) d -> two f d", two=2))
        z2g = p.zg.tile([P, D], f32, tag=f"z2g{grp}", name=f"z2g_{grp}")
        ldma.dma_start(z2g[:], z2[b0:b0 + P, :].rearrange(
            "(f two) d -> two f d", two=2))
        zs[grp] = (z1g, z2g)
    return zs


def _cast_zh(p, nc, zs, grp):
    """Plain fp16 casts of z1/z2 into ONE tile (no stats dependency)."""
    z1g, z2g = zs[grp]
    zh = p.zh.tile([P, 2 * D], f16, tag=f"zh{grp}", name=f"zh_{grp}")
    nc.scalar.copy(zh[:, 0:D], z1g[:])
    nc.scalar.copy(zh[:, D:2 * D], z2g[:])
    return zh


def _emit_zi(p, nc, zh, grp):
    """lhsT gather: one 2-partition DMA per interleaved half."""
    ZI = p.zi.tile([P, HF * D], f16, tag="ZI", name=f"ZI_{grp}")
    dmae = nc.sync if grp == 0 else nc.gpsimd
    for half in range(2):
        hb, pp = HF * half, 32 * half
        dmae.dma_start(ZI[pp:pp + 1, :], zh[hb:hb + HF, 0:D])
        dmae.dma_start(ZI[pp + 1:pp + 2, :], zh[hb:hb + HF, D:2 * D])
    return ZI


def _emit_ri(p, nc, Rall, grp):
    """rhs gather. Group 0 is split into prefix waves so chunk 0's first
    matmuls only wait on a tiny first DMA (Tile tracks write ranges), the
    rest follows while they run. Later groups: one DMA per row."""
    RI = p.ri.tile([P, HF * KD], f16, tag="RI", name=f"RI_{grp}")
    dmae = nc.sync if grp == 0 else nc.gpsimd
    waves = [(0, 12), (12, 32), (32, HF)] if grp == 0 else [(0, HF)]
    for lo, hi in waves:
        for half in range(2):
            hb, pp = HF * half, 32 * half
            dmae.dma_start(RI[pp:pp + 1, lo * KD:hi * KD],
                           Rall[hb + lo:hb + hi, 0:KD])
            dmae.dma_start(RI[pp + 1:pp + 2, lo * KD:hi * KD],
                           Rall[hb + lo:hb + hi, KD:2 * KD])
    return RI


def _stats_norms_tt(p, nc, zs, grp):
    """Elementwise squares/products (on the group's stats engine)."""
    gps = grp >= 2
    eng = nc.gpsimd if gps else nc.vector
    z1g, z2g = zs[grp]

    def wt(tag):
        return p.work.tile([P, D], f32, tag=tag, name=f"w_{tag}_{grp}")

    def sv(tag):
        return p.stat.tile([P, 1], f32, tag=tag, name=f"sv_{tag}_{grp}")

    def nv(tag, wide=False):
        return p.nrm.tile([P, D if wide else 1], f32, tag=f"{tag}{grp}",
                          name=f"n_{tag}_{grp}")

    st = {"eng": eng, "gps": gps, "wt": wt, "sv": sv, "nv": nv,
          "z1g": z1g, "z2g": z2g}
    v1z, v2z, wz = nv("v1z", True), nv("v2z", True), nv("wz", True)
    eng.tensor_mul(v1z[:], z1g[:], z1g[:])
    eng.tensor_mul(v2z[:], z2g[:], z2g[:])
    eng.tensor_mul(wz[:], z1g[:], z2g[:])
    st.update(v1z=v1z, v2z=v2z, wz=wz)
    return st


def _stats_norms_fin(p, nc, st, grp):
    """Reduces/reciprocals (DVE-only) + ab2t on the stats engine + ACT sqrt.
    Emitted upfront for ALL groups: the sqrt hops land on ACT before the
    copy stream starts, so they never stall copies mid-kernel."""
    eng = st["eng"]
    nv = st["nv"]
    v1z, v2z, wz = st["v1z"], st["v2z"], st["wz"]
    s1, s2, dot = nv("s1"), nv("s2"), nv("dot")
    nc.vector.reduce_sum(s1[:], v1z[:], axis=mybir.AxisListType.X)
    nc.vector.reduce_sum(s2[:], v2z[:], axis=mybir.AxisListType.X)
    nc.vector.reduce_sum(dot[:], wz[:], axis=mybir.AxisListType.X)
    mz1, mz2 = nv("mz1"), nv("mz2")
    nc.vector.reduce_max(mz1[:], st["z1g"][:], axis=mybir.AxisListType.X,
                         apply_absolute_value=True)
    nc.vector.reduce_max(mz2[:], st["z2g"][:], axis=mybir.AxisListType.X,
                         apply_absolute_value=True)
    a2, b2 = nv("a2"), nv("b2")
    nc.vector.reciprocal(a2[:], s1[:])
    nc.vector.reciprocal(b2[:], s2[:])
    ab2t, ab = nv("ab2t"), nv("ab")
    eng.tensor_mul(ab2t[:], a2[:], b2[:])
    nc.scalar.sqrt(ab[:], ab2t[:])
    st.update(s1=s1, s2=s2, dot=dot, mz1=mz1, mz2=mz2, a2=a2, b2=b2,
              ab2t=ab2t, ab=ab)
    return st


def _stats_scale(p, nc, st, grp):
    """int8 bound -> s = 126/bound, plus the s-scaled coefficient set.

    Bound (valid upper bound on max_k |H_k_rank2[i,j]|, using |c|<=1 and
    mu*mv <= (mu^2+mv^2)/2, ab <= (a2+b2)/2):
      pp = (mz1*a)^2, qq = (mz2*b)^2, pq = pp+qq
      bound = a2*(3pp+pq) + 0.75*(a2+b2)*pq + b2*(3qq+pq)
    """
    A = mybir.AluOpType
    gps = st["gps"]
    eng = st["eng"]
    sv = st["sv"]
    mz1, mz2, a2, b2, ab = st["mz1"], st["mz2"], st["a2"], st["b2"], st["ab"]
    mz1s, mz2s, pp_, qq, pq = (sv("mz1s"), sv("mz2s"), sv("pp"), sv("qq"),
                               sv("pq"))
    eng.tensor_mul(mz1s[:], mz1[:], mz1[:])
    eng.tensor_mul(mz2s[:], mz2[:], mz2[:])
    eng.tensor_mul(pp_[:], mz1s[:], a2[:])
    eng.tensor_mul(qq[:], mz2s[:], b2[:])
    eng.tensor_add(pq[:], pp_[:], qq[:])
    t11, s11, b11 = sv("t11"), sv("s11"), sv("b11")
    t22, s22, b22 = sv("t22"), sv("s22"), sv("b22")
    hh, s12, b12 = sv("hh"), sv("s12"), sv("b12")
    if gps:
        eng.tensor_mul(t11[:], pp_[:], p.c3[:])
        eng.tensor_mul(t22[:], qq[:], p.c3[:])
    else:
        eng.tensor_scalar(t11[:], pp_[:], 3.0, None, A.mult)
        eng.tensor_scalar(t22[:], qq[:], 3.0, None, A.mult)
    eng.tensor_add(s11[:], t11[:], pq[:])
    eng.tensor_mul(b11[:], s11[:], a2[:])
    eng.tensor_add(s22[:], t22[:], pq[:])
    eng.tensor_mul(b22[:], s22[:], b2[:])
    eng.tensor_add(hh[:], a2[:], b2[:])
    eng.tensor_mul(s12[:], pq[:], hh[:])
    if gps:
        eng.tensor_mul(b12[:], s12[:], p.c075[:])
    else:
        eng.tensor_scalar(b12[:], s12[:], 0.75, None, A.mult)
    bs, inv, s = sv("bs"), sv("inv"), sv("s")
    bound = st["nv"]("bound")
    eng.tensor_add(bs[:], b11[:], b12[:])
    eng.tensor_add(bound[:], bs[:], b22[:])
    st["bound"] = bound
    return st


def _stats_coef(p, nc, st, grp):
    """reciprocal (DVE) + s + the s-scaled coefficient chain. For GPSIMD
    groups this is emitted in-loop AFTER the bound chain has long completed,
    so the lone DVE reciprocal never stalls the copy queue."""
    A = mybir.AluOpType
    gps = st["gps"]
    eng = st["eng"]
    sv = st["sv"]
    a2, b2, ab, bound = st["a2"], st["b2"], st["ab"], st["bound"]
    inv, s = sv("inv"), sv("s")
    nc.vector.reciprocal(inv[:], bound[:])
    if gps:
        eng.tensor_mul(s[:], inv[:], p.c126[:])
    else:
        eng.tensor_scalar(s[:], inv[:], 126.0, None, A.mult)

    # ---- coefficient chain (c, diag helpers, s-scaled R coefficients) ----
    nv = st["nv"]
    c, m3c, mcab = nv("c"), nv("m3c"), nv("mcab")
    mc = sv("mc")
    eng.tensor_mul(c[:], st["dot"][:], ab[:])
    if gps:
        eng.tensor_mul(m3c[:], c[:], p.cm3[:])
        eng.tensor_mul(mc[:], c[:], p.cm1[:])
    else:
        eng.tensor_scalar(m3c[:], c[:], -3.0, None, A.mult)
        eng.tensor_scalar(mc[:], c[:], -1.0, None, A.mult)
    eng.tensor_mul(mcab[:], mc[:], ab[:])
    A3B, AB3, A4, B4, A2B2 = (sv("A3B"), sv("AB3"), sv("A4"), sv("B4"),
                              sv("A2B2"))
    eng.tensor_mul(A3B[:], a2[:], ab[:])
    eng.tensor_mul(AB3[:], b2[:], ab[:])
    eng.tensor_mul(A4[:], a2[:], a2[:])
    eng.tensor_mul(B4[:], b2[:], b2[:])
    eng.tensor_mul(A2B2[:], ab[:], ab[:])
    cs, m3cs, mcs = sv("cs"), sv("m3cs"), sv("mcs")
    eng.tensor_mul(cs[:], c[:], s[:])
    if gps:
        eng.tensor_mul(m3cs[:], cs[:], p.cm3[:])
        eng.tensor_mul(mcs[:], cs[:], p.cm1[:])
    else:
        eng.tensor_scalar(m3cs[:], cs[:], -3.0, None, A.mult)
        eng.tensor_scalar(mcs[:], cs[:], -1.0, None, A.mult)
    alv, bev, gav, dev, epv = (sv("alv"), sv("bev"), sv("gav"), sv("dev"),
                               sv("epv"))
    eng.tensor_mul(alv[:], m3cs[:], A4[:])
    eng.tensor_mul(bev[:], A3B[:], s[:])
    eng.tensor_mul(gav[:], mcs[:], A2B2[:])
    eng.tensor_mul(dev[:], AB3[:], s[:])
    eng.tensor_mul(epv[:], m3cs[:], B4[:])
    st.update(c=c, m3c=m3c, mcab=mcab,
              alv=alv, bev=bev, gav=gav, dev=dev, epv=epv)
    return st


def _stats_phase1(p, nc, st, grp):
    """R rows in fp16, s pre-folded, packed in ONE tile:
    Rall = [R0 | R1] = [A | B | Dd  |  B | C | E] with
      A = alv*z1 + bev*z2   B = bev*z1   Dd = dev*z2
      C = gav*z1 + dev*z2   E = dev*z1 + epv*z2
    """
    A = mybir.AluOpType
    gps = st["gps"]
    eng = st["eng"]
    z1g, z2g, wt = st["z1g"], st["z2g"], st["wt"]
    Rall = p.rall.tile([P, 2 * KD], f16, tag="Rall", name=f"Rall_{grp}")

    def ts(out, in0, svt):
        if gps:
            eng.tensor_mul(out, in0, svt[:].broadcast_to([P, in0.shape[-1]]))
        else:
            eng.tensor_scalar(out, in0, svt[:], None, A.mult)

    def stt(out, in0, svt, in1, tag):
        if gps:
            w = in0.shape[-1]
            tmp = wt(tag)
            eng.tensor_mul(tmp[:, 0:w], in0,
                           svt[:].broadcast_to([P, w]))
            eng.tensor_add(out, tmp[:, 0:w], in1)
        else:
            eng.scalar_tensor_tensor(out, in0, svt[:], in1, A.mult, A.add)

    alv, bev, gav, dev, epv = (st["alv"], st["bev"], st["gav"], st["dev"],
                               st["epv"])
    t0 = wt("t0")
    ts(t0[:], z2g[:], bev)
    stt(Rall[:, 0:D], z1g[:], alv, t0[:], "p1a")           # A
    ts(Rall[:, D:2 * D], z1g[:], bev)                      # B
    ts(Rall[:, 2 * D:3 * D], z2g[:], dev)                  # Dd
    ts(Rall[:, 3 * D:4 * D], z1g[:], bev)                  # B (R1 copy)
    t1 = wt("t1")
    ts(t1[:], z2g[:], dev)
    stt(Rall[:, 4 * D:5 * D], z1g[:], gav, t1[:], "p1b")   # C
    t2 = wt("t2")
    ts(t2[:], z2g[:], epv)
    stt(Rall[:, 5 * D:6 * D], z1g[:], dev, t2[:], "p1c")   # E
    st["Rall"] = Rall


def _stats_phase2(p, nc, st, grp, diag_hw):
    """Final diagonal values, batch-major [128b, 3*128i]; DMA'd out as f32.
    ALWAYS on GPSIMD (latency-irrelevant; keeps DVE/ACT for copies).
    Host overwrites out[k, b, i, i] with these.
    """
    eng = nc.gpsimd
    wt = st["wt"]

    def sv(tag):
        return p.stat.tile([P, 1], f32, tag=tag, name=f"sv_{tag}_{grp}")

    v1z, v2z, wz = st["v1z"], st["v2z"], st["wz"]
    a2, b2, ab, c, m3c = st["a2"], st["b2"], st["ab"], st["c"], st["m3c"]
    dall = p.dpool.tile([P, KD], f32, tag="dall", name=f"dall_{grp}")

    def bc(svt):
        return svt[:].broadcast_to([P, D])

    twoabw = wt("twoabw")
    ab2 = sv("ab2")
    eng.tensor_add(ab2[:], ab[:], ab[:])
    eng.tensor_mul(twoabw[:], wz[:], bc(ab2))
    # d11 = a2*(c + 2ab*wz + m3c*a2*v1z)
    u1, u2, u2c = wt("u1"), wt("u2"), wt("u2c")
    pa = sv("pa")
    eng.tensor_mul(pa[:], a2[:], m3c[:])
    eng.tensor_mul(u1[:], v1z[:], bc(pa))
    eng.tensor_add(u2[:], u1[:], twoabw[:])
    eng.tensor_add(u2c[:], u2[:], bc(c))
    eng.tensor_mul(dall[:, 0:D], u2c[:], bc(a2))
    # d12 = ab*(a2*v1z + b2*v2z + mcab*wz - 1)
    w1, w2, w2b, w3, w3b, w4 = (wt("w1"), wt("w2"), wt("w2b"), wt("w3"),
                                wt("w3b"), wt("w4"))
    eng.tensor_mul(w1[:], v1z[:], bc(a2))
    eng.tensor_mul(w2[:], v2z[:], bc(b2))
    eng.tensor_add(w2b[:], w2[:], w1[:])
    eng.tensor_mul(w3[:], wz[:], bc(st["mcab"]))
    eng.tensor_add(w3b[:], w3[:], w2b[:])
    eng.tensor_mul(w4[:], w3b[:], bc(ab))
    eng.tensor_sub(dall[:, D:2 * D], w4[:], bc(ab))
    # d22 = b2*(c + 2ab*wz + m3c*b2*v2z)
    u3, u4, u4c = wt("u3"), wt("u4"), wt("u4c")
    pb = sv("pb")
    eng.tensor_mul(pb[:], b2[:], m3c[:])
    eng.tensor_mul(u3[:], v2z[:], bc(pb))
    eng.tensor_add(u4[:], u3[:], twoabw[:])
    eng.tensor_add(u4c[:], u4[:], bc(c))
    eng.tensor_mul(dall[:, 2 * D:3 * D], u4c[:], bc(b2))
    nc.gpsimd.dma_start(diag_hw[grp], dall[:])
    # dequant scale (bound) to DRAM; deferred here so it never blocks the
    # GPSIMD queue head during the ramp
    nc.gpsimd.dma_start(p.scl_hw[grp], st["bound"][:])


def _emit_chunk(p, nc, ZI, RI, out_hw, grp, ch, qctr):
    """G elements (matmul + quad PSUM->int8 copy) + output DMA."""
    e0 = grp * P + ch * G     # global element base for this chunk
    ci = grp * NCH + ch
    GS = 8 if ci in (0, GROUPS * NCH - 1) else 16
    for sub in range(G // GS):
        STG = p.stage.tile([P, GS * KD], i8, tag="STG",
                           name=f"STG_{grp}_{ch}_{sub}")
        for t in range(GS // QB):
            pt = p.mmp.tile([P, QB * 512], f32, tag="pt",
                            name=f"pt_{grp}_{ch}_{sub}_{t}")
            for slot in range(QB):
                s = sub * GS + t * QB + slot   # local element in chunk
                half = s % 2                   # PE quadrant alternation
                ff = ch * (G // 2) + s // 2    # column within group tile
                pp = 32 * half
                lhsT = ZI[pp:pp + 2, ff * D:(ff + 1) * D]
                rhs = RI[pp:pp + 2, ff * KD:(ff + 1) * KD]
                nc.tensor.matmul(pt[:, slot * 512:slot * 512 + KD],
                                 lhsT, rhs, start=True, stop=True)
            src = pt[:].rearrange("p (q c) -> p q c", c=512)[:, :, 0:KD]
            dst = STG[:, t * QB * KD:(t + 1) * QB * KD].rearrange(
                "p (q c) -> p q c", c=KD)
            if _copy_engine(qctr[0]):
                nc.scalar.copy(dst, src)
            else:
                nc.vector.tensor_copy(dst, src)
            qctr[0] += 1
        es = e0 + sub * GS
        nc.sync.dma_start(out_hw[:, es * KD:(es + GS) * KD], STG[:])


def _build_body(ctx, tc, z1, z2, out_hw, diag_hw, scl_hw):
    nc = tc.nc
    p = _make_pools(ctx, tc)
    p.scl_hw = scl_hw
    _make_consts(p, nc)

    qctr = [0]
    zs = _load_all_z(p, nc, z1, z2)
    # z fp16 casts + lhsT gathers have NO stats dependency -> fire at t=0
    zh = {0: _cast_zh(p, nc, zs, 0), 1: _cast_zh(p, nc, zs, 1)}
    gzi = {0: _emit_zi(p, nc, zh[0], 0), 1: _emit_zi(p, nc, zh[1], 1)}
    # group 0-1 stats on DVE during the ramp window
    # group 0 chain on DVE: the ramp-critical path
    sts = {0: _stats_norms_tt(p, nc, zs, 0)}
    _stats_norms_fin(p, nc, sts[0], 0)
    _stats_scale(p, nc, sts[0], 0)
    _stats_coef(p, nc, sts[0], 0)
    _stats_phase1(p, nc, sts[0], 0)
    gri = {0: _emit_ri(p, nc, sts[0]["Rall"], 0)}
    # groups 2-3 norms upfront (GPSIMD TTs + DVE reduces + ACT sqrts): their
    # sqrt hops run on ACT BEFORE the copy stream starts, never stalling it
    sts[2] = _stats_norms_tt(p, nc, zs, 2)
    sts[3] = _stats_norms_tt(p, nc, zs, 3)
    _stats_norms_fin(p, nc, sts[2], 2)
    _stats_norms_fin(p, nc, sts[3], 3)
    # group 1 chain on DVE
    sts[1] = _stats_norms_tt(p, nc, zs, 1)
    _stats_norms_fin(p, nc, sts[1], 1)
    _stats_scale(p, nc, sts[1], 1)
    _stats_coef(p, nc, sts[1], 1)
    _stats_phase1(p, nc, sts[1], 1)
    # groups 2-3 bound chains: pure GPSIMD, all inputs ready -> run in the
    # ramp window, emitted BEFORE RI1 whose Rall1-wait blocks the GPSIMD
    # queue head until ~32us
    _stats_scale(p, nc, sts[2], 2)
    _stats_scale(p, nc, sts[3], 3)
    # ... and their coef chains: the lone DVE reciprocal lands in the DVE
    # FIFO right after group 1's chain (inputs ready, no stall), instead of
    # anchoring the whole g2/g3 R-chain to a mid-copy-stream FIFO slot
    _stats_coef(p, nc, sts[2], 2)
    _stats_coef(p, nc, sts[3], 3)
    gri[1] = _emit_ri(p, nc, sts[1]["Rall"], 1)
    # diag work entirely on GPSIMD, never latency-critical
    _stats_phase2(p, nc, sts[0], 0, diag_hw)
    _stats_phase2(p, nc, sts[1], 1, diag_hw)
    zh[2] = _cast_zh(p, nc, zs, 2)
    zh[3] = _cast_zh(p, nc, zs, 3)
    TOT = GROUPS * NCH
    for ci in range(TOT):
        grp, ch = divmod(ci, NCH)
        if ch == 0 and grp in (1, 2):
            # groups 2-3: gathers 3.5 chunks ahead of first use, emitted as
            # soon as the previous group's chunks (the buffer's last readers)
            # are all emitted, so the GPSIMD queue has slack to deliver them
            gzi[grp + 1] = _emit_zi(p, nc, zh[grp + 1], grp + 1)
            gri[grp + 1] = _emit_ri(p, nc, sts[grp + 1]["Rall"], grp + 1)
        _emit_chunk(p, nc, gzi[grp], gri[grp], out_hw, grp, ch, qctr)
        # remaining stats for group g+2 on GPSIMD (norms already done in the
        # ramp window; only the GPS-local coef/R/diag chains remain)
        if grp + 2 < GROUPS:
            if ch == 2:
                _stats_phase1(p, nc, sts[grp + 2], grp + 2)
            elif ch == 3:
                _stats_phase2(p, nc, sts[grp + 2], grp + 2, diag_hw)


def build_kernel():
    nc = bacc.Bacc("TRN2", target_bir_lowering=False, debug=False)
    z1 = nc.dram_tensor("z1", [B_SH, D], f32, kind="ExternalInput").ap()
    z2 = nc.dram_tensor("z2", [B_SH, D], f32, kind="ExternalInput").ap()
    # packed rank-2 output: [i partition, (b, k, j) free], int8
    out_hw = nc.dram_tensor("out", [P, B_SH * KD], i8,
                            kind="ExternalOutput").ap()
    scl_hw = nc.dram_tensor("scl", [GROUPS, P, 1], f32,
                            kind="ExternalOutput").ap()
    # final diagonals: [group, b-partition (interleaved), (k, i) free], f32
    diag_hw = nc.dram_tensor("diag", [GROUPS, P, KD], f32,
                             kind="ExternalOutput").ap()
    with tile.TileContext(nc) as tc:
        with ExitStack() as ctx:
            _build_body(ctx, tc, z1, z2, out_hw, diag_hw, scl_hw)
    nc.compile()
    return nc


_NC_CACHE = None


def _get_nc():
    global _NC_CACHE
    if _NC_CACHE is None:
        _NC_CACHE = build_kernel()
    return _NC_CACHE


def _assemble(out_c, diag_c, scl_c, dst):
    """Unpack one core's HW tensors into dst [3, B_SH, D, D] f32."""
    # out_c [128 i, B_SH*384] int8 -> [i, b, k, j] -> [k, b, i, j]; dequant
    # with the per-element scale bound/126 (scl rows are partition-ordered)
    h = out_c.reshape(P, B_SH, 3, D).transpose(2, 1, 0, 3)
    scale = (scl_c.reshape(GROUPS, P)[:, PART_OF_ELEM].reshape(B_SH)
             / np.float32(126.0))
    np.multiply(h, scale[None, :, None, None], out=dst, dtype=np.float32)
    # diag_c [GROUPS, 128 p, 3*128] f32; partition p holds group element
    # E_OF_P... i.e. ordered element e sits at partition P_OF_E[e]
    dv = diag_c[:, PART_OF_ELEM, :].reshape(GROUPS, P, 3, D).transpose(
        2, 0, 1, 3).reshape(3, B_SH, D)
    dst.reshape(3, B_SH, D * D)[:, :, ::D + 1] = dv


def kernel(z1, z2):
    nc = _get_nc()
    z1 = np.ascontiguousarray(np.asarray(z1, dtype=np.float32))
    z2 = np.ascontiguousarray(np.asarray(z2, dtype=np.float32))
    in_maps = [
        {"z1": z1[c * B_SH:(c + 1) * B_SH], "z2": z2[c * B_SH:(c + 1) * B_SH]}
        for c in range(N_CORES)
    ]
    res = run_bass_kernel_spmd(nc, in_maps, core_ids=list(range(N_CORES)))
    full = np.empty((3, B_FULL, D, D), dtype=np.float32)
    for c in range(N_CORES):
        _assemble(res.results[c]["out"], res.results[c]["diag"],
                  res.results[c]["scl"], full[:, c * B_SH:(c + 1) * B_SH])
    return full


# revision 44
# speedup vs baseline: 1.0116x; 1.0116x over previous
"""Trainium2 Bass kernel for ArccosHessianCalculator (int8-packed output).

Math: for each batch element b (z1, z2 are [B, D] with D = 128):
  a = 1/|z1|, bb = 1/|z2|, c = cos = <z1u, z2u>
  Each Hessian block H_k is rank-2 plus a diagonal term:
      H_k(b) = z1 * r0_k(b)^T + z2 * r1_k(b)^T + diag
  with the normalization / cosine factors folded into r0/r1:
      k=0 (H11): r0 = -3c*a^4*z1 + a^3 b*z2          r1 = a^3 b*z1
      k=1 (H12): r0 = a^3 b*z1                        r1 = -c*a^2 b^2*z1 + a b^3*z2
      k=2 (H22): r0 = a b^3*z2                        r1 = a b^3*z1 - 3c*b^4*z2

Device strategy (per core, batch shard of 512):
  - TensorE: one K=2 matmul per element, lhsT = [z1(b); z2(b)] ([2,128] fp16),
    rhs = [r0 | r1] blocks ([2,384] fp16) -> PSUM f32 [128, 384]. Elements
    alternate PE quadrant rows ({0,1} / {32,33}) so LDWEIGHTS for element
    e+1 overlaps the stream of element e.
  - INT8 output: a guaranteed per-element bound on |H_rank2| (triangle
    inequality with |c|<=1 and AM-GM, ~16 [128,1] ops) gives s = 126/bound.
    s is folded ENTIRELY into the R coefficients, so the z fp16 casts have
    NO stats dependency (lhsT gathers fire at t=0) and PSUM holds s*H; the
    PSUM->SBUF copy is a plain int8 cast. The rank-2 part lands in DRAM
    packed as out_hw [128 i, B_SH*384 (b,k,j)] int8 -- 4x less HBM traffic
    than f32.
  - Final diagonals are computed in closed form per group ([128 b, 3*128 i])
    into a small f32 side tensor and spliced on the HOST in f32 (they
    dominate the Hessian's magnitude, so they carry no quantization error).
  - PSUM: 2-bank super-tiles x4 buffers; each drained by ONE strided copy
    (2 elements) split 5:6 between DVE and ACT (weighted by their measured
    copy rates; PSUM reads are f32 at ~1 elem/cycle/partition).
  - Stats: group 0-1 chains on DVE (short ramp); groups 2-3 and ALL diag
    (phase2) work on GPSIMD, which is otherwise idle, so recurring stats
    never steal copy throughput. Free-axis reduces / reciprocal stay on DVE
    (GPSIMD cannot do them), sqrt on ACT. All z tiles prefetch at t=0.
  - Batch rows are loaded interleaved (even elements -> partitions 0..63,
    odd -> 64..127) so gathers are plain partition slices; z and R live in
    combined tiles (zh [128,256], Rall [128,768]) so each gather half is a
    single 2-partition DMA (4 dispatches per group instead of 8).
  - Host: reshape/transpose view + per-element dequant multiply + f32
    diagonal stride-trick splice.
"""

import numpy as np
from contextlib import ExitStack

import concourse.bass as bass
import concourse.tile as tile
from concourse import bacc, mybir
from concourse.bass_utils import run_bass_kernel_spmd

N_CORES = 8
B_FULL = 4096
D = 128
B_SH = B_FULL // N_CORES  # 512 batch elements per core
P = 128                   # SBUF partitions
KD = 3 * D                # 384: three H blocks side by side
G = 32                    # elements per chunk
GROUPS = B_SH // P        # 4 stats groups of 128 elements
NCH = P // G              # 4 chunks per group
QB = 2                    # elements per PSUM super-tile (2 banks)
OC = 320                  # output cols per element (256 main + 64 packed corners)
HF = P // 2               # 64 elements per interleaved half

f32 = mybir.dt.float32
f16 = mybir.dt.float16
i8 = mybir.dt.int8

# interleaved element order within a group: partition p holds group element
# 2p (p < 64) or 2(p-64)+1 (p >= 64)
ELEM_OF_PART = np.concatenate([np.arange(0, P, 2), np.arange(1, P, 2)])
PART_OF_ELEM = np.argsort(ELEM_OF_PART)

# copy-engine schedule: 6 ACT : 5 DVE interleaved (ACT is ~1.18x faster at
# PSUM->SBUF f32 copies: (172+FD)/1.2GHz vs (120+FD)/0.96GHz).
# The first 18 super-tiles run 2:1 ACT-heavy: DVE is still draining the
# group-1 stats chain during the ramp-out.
COPY_PAT = [1, 0, 1, 0, 1, 0, 1, 0, 1, 0, 1]  # 1 = ACT


def _copy_engine(q):
    if q < 18:
        return 1 if q % 3 != 2 else 0
    return COPY_PAT[q % len(COPY_PAT)]


class _Pools:
    pass


def _make_pools(ctx, tc):
    p = _Pools()
    p.const = ctx.enter_context(tc.tile_pool(name="const", bufs=1))
    p.zg = ctx.enter_context(tc.tile_pool(name="zg", bufs=1))
    p.zh = ctx.enter_context(tc.tile_pool(name="zh", bufs=4))
    p.work = ctx.enter_context(tc.tile_pool(name="work", bufs=2))
    p.nrm = ctx.enter_context(tc.tile_pool(name="nrm", bufs=1))
    p.stat = ctx.enter_context(tc.tile_pool(name="stat", bufs=3))
    p.rall = ctx.enter_context(tc.tile_pool(name="rall", bufs=2))
    p.dpool = ctx.enter_context(tc.tile_pool(name="dpool", bufs=2))
    p.zi = ctx.enter_context(tc.tile_pool(name="zi", bufs=2))
    p.ri = ctx.enter_context(tc.tile_pool(name="ri", bufs=2))
    p.stage = ctx.enter_context(tc.tile_pool(name="stage", bufs=5))
    p.mmp = ctx.enter_context(tc.tile_pool(name="mmp", bufs=4, space="PSUM"))
    return p


def _make_consts(p, nc):
    """[128,1] constant tiles for GPSIMD (Pool has no scalar immediates)."""
    for tag, val in (("c3", 3.0), ("c075", 0.75), ("c126", 126.0),
                     ("cm1", -1.0), ("cm3", -3.0)):
        t = p.const.tile([P, 1], f32, tag=tag, name=tag)
        nc.vector.memset(t[:], val)
        setattr(p, tag, t)


def _load_all_z(p, nc, z1, z2):
    """Prefetch every group's z tiles at t=0 (tiny: 512B/partition each)."""
    zs = {}
    for grp in range(GROUPS):
        b0 = grp * P
        ldma = nc.sync if grp == 0 else nc.gpsimd
        # interleaved row order: partition p <- batch row 2p / 2(p-64)+1
        z1g = p.zg.tile([P, D], f32, tag=f"z1g{grp}", name=f"z1g_{grp}")
        ldma.dma_start(z1g[:], z1[b0:b0 + P, :].rearrange(
            "(f two) d -> two f d", two=2))
        z2g = p.zg.tile([P, D], f32, tag=f"z2g{grp}", name=f"z2g_{grp}")
        ldma.dma_start(z2g[:], z2[b0:b0 + P, :].rearrange(
            "(f two) d -> two f d", two=2))
        zs[grp] = (z1g, z2g)
    return zs


def _cast_zh(p, nc, zs, grp):
    """Plain fp16 casts of z1/z2 into ONE tile (no stats dependency)."""
    z1g, z2g = zs[grp]
    zh = p.zh.tile([P, 2 * D], f16, tag=f"zh{grp}", name=f"zh_{grp}")
    nc.scalar.copy(zh[:, 0:D], z1g[:])
    nc.scalar.copy(zh[:, D:2 * D], z2g[:])
    return zh


def _emit_zi(p, nc, zh, grp):
    """lhsT gather: one 2-partition DMA per interleaved half."""
    ZI = p.zi.tile([P, HF * D], f16, tag="ZI", name=f"ZI_{grp}")
    dmae = nc.sync if grp == 0 else nc.gpsimd
    for half in range(2):
        hb, pp = HF * half, 32 * half
        dmae.dma_start(ZI[pp:pp + 1, :], zh[hb:hb + HF, 0:D])
        dmae.dma_start(ZI[pp + 1:pp + 2, :], zh[hb:hb + HF, D:2 * D])
    return ZI


def _emit_ri(p, nc, Rall, grp):
    """rhs gather. Group 0 is split into prefix waves so chunk 0's first
    matmuls only wait on a tiny first DMA (Tile tracks write ranges), the
    rest follows while they run. Later groups: one DMA per row."""
    RI = p.ri.tile([P, HF * KD], f16, tag="RI", name=f"RI_{grp}")
    dmae = nc.sync if grp == 0 else nc.gpsimd
    waves = [(0, 12), (12, 32), (32, HF)] if grp == 0 else [(0, HF)]
    for lo, hi in waves:
        for half in range(2):
            hb, pp = HF * half, 32 * half
            dmae.dma_start(RI[pp:pp + 1, lo * KD:hi * KD],
                           Rall[hb + lo:hb + hi, 0:KD])
            dmae.dma_start(RI[pp + 1:pp + 2, lo * KD:hi * KD],
                           Rall[hb + lo:hb + hi, KD:2 * KD])
    return RI


def _stats_norms_tt(p, nc, zs, grp):
    """Elementwise squares/products (on the group's stats engine)."""
    gps = grp >= 2
    eng = nc.gpsimd if gps else nc.vector
    z1g, z2g = zs[grp]

    def wt(tag):
        return p.work.tile([P, D], f32, tag=tag, name=f"w_{tag}_{grp}")

    def sv(tag):
        return p.stat.tile([P, 1], f32, tag=tag, name=f"sv_{tag}_{grp}")

    def nv(tag, wide=False):
        return p.nrm.tile([P, D if wide else 1], f32, tag=f"{tag}{grp}",
                          name=f"n_{tag}_{grp}")

    st = {"eng": eng, "gps": gps, "wt": wt, "sv": sv, "nv": nv,
          "z1g": z1g, "z2g": z2g}
    v1z, v2z, wz = nv("v1z", True), nv("v2z", True), nv("wz", True)
    eng.tensor_mul(v1z[:], z1g[:], z1g[:])
    eng.tensor_mul(v2z[:], z2g[:], z2g[:])
    eng.tensor_mul(wz[:], z1g[:], z2g[:])
    st.update(v1z=v1z, v2z=v2z, wz=wz)
    return st


def _stats_norms_fin(p, nc, st, grp):
    """Reduces/reciprocals (DVE-only) + ab2t on the stats engine + ACT sqrt.
    Emitted upfront for ALL groups: the sqrt hops land on ACT before the
    copy stream starts, so they never stall copies mid-kernel."""
    eng = st["eng"]
    nv = st["nv"]
    v1z, v2z, wz = st["v1z"], st["v2z"], st["wz"]
    s1, s2, dot = nv("s1"), nv("s2"), nv("dot")
    nc.vector.reduce_sum(s1[:], v1z[:], axis=mybir.AxisListType.X)
    nc.vector.reduce_sum(s2[:], v2z[:], axis=mybir.AxisListType.X)
    nc.vector.reduce_sum(dot[:], wz[:], axis=mybir.AxisListType.X)
    mz1, mz2 = nv("mz1"), nv("mz2")
    nc.vector.reduce_max(mz1[:], st["z1g"][:], axis=mybir.AxisListType.X,
                         apply_absolute_value=True)
    nc.vector.reduce_max(mz2[:], st["z2g"][:], axis=mybir.AxisListType.X,
                         apply_absolute_value=True)
    a2, b2 = nv("a2"), nv("b2")
    nc.vector.reciprocal(a2[:], s1[:])
    nc.vector.reciprocal(b2[:], s2[:])
    ab2t, ab = nv("ab2t"), nv("ab")
    eng.tensor_mul(ab2t[:], a2[:], b2[:])
    nc.scalar.sqrt(ab[:], ab2t[:])
    st.update(s1=s1, s2=s2, dot=dot, mz1=mz1, mz2=mz2, a2=a2, b2=b2,
              ab2t=ab2t, ab=ab)
    return st


def _stats_scale(p, nc, st, grp):
    """int8 bound -> s = 126/bound, plus the s-scaled coefficient set.

    Bound (valid upper bound on max_k |H_k_rank2[i,j]|, using |c|<=1 and
    mu*mv <= (mu^2+mv^2)/2, ab <= (a2+b2)/2):
      pp = (mz1*a)^2, qq = (mz2*b)^2, pq = pp+qq
      bound = a2*(3pp+pq) + 0.75*(a2+b2)*pq + b2*(3qq+pq)
    """
    A = mybir.AluOpType
    gps = st["gps"]
    eng = st["eng"]
    sv = st["sv"]
    mz1, mz2, a2, b2, ab = st["mz1"], st["mz2"], st["a2"], st["b2"], st["ab"]
    mz1s, mz2s, pp_, qq, pq = (sv("mz1s"), sv("mz2s"), sv("pp"), sv("qq"),
                               sv("pq"))
    eng.tensor_mul(mz1s[:], mz1[:], mz1[:])
    eng.tensor_mul(mz2s[:], mz2[:], mz2[:])
    eng.tensor_mul(pp_[:], mz1s[:], a2[:])
    eng.tensor_mul(qq[:], mz2s[:], b2[:])
    eng.tensor_add(pq[:], pp_[:], qq[:])
    t11, s11, b11 = sv("t11"), sv("s11"), sv("b11")
    t22, s22, b22 = sv("t22"), sv("s22"), sv("b22")
    hh, s12, b12 = sv("hh"), sv("s12"), sv("b12")
    if gps:
        eng.tensor_mul(t11[:], pp_[:], p.c3[:])
        eng.tensor_mul(t22[:], qq[:], p.c3[:])
    else:
        eng.tensor_scalar(t11[:], pp_[:], 3.0, None, A.mult)
        eng.tensor_scalar(t22[:], qq[:], 3.0, None, A.mult)
    eng.tensor_add(s11[:], t11[:], pq[:])
    eng.tensor_mul(b11[:], s11[:], a2[:])
    eng.tensor_add(s22[:], t22[:], pq[:])
    eng.tensor_mul(b22[:], s22[:], b2[:])
    eng.tensor_add(hh[:], a2[:], b2[:])
    eng.tensor_mul(s12[:], pq[:], hh[:])
    if gps:
        eng.tensor_mul(b12[:], s12[:], p.c075[:])
    else:
        eng.tensor_scalar(b12[:], s12[:], 0.75, None, A.mult)
    bs, inv, s = sv("bs"), sv("inv"), sv("s")
    bound = st["nv"]("bound")
    eng.tensor_add(bs[:], b11[:], b12[:])
    eng.tensor_add(bound[:], bs[:], b22[:])
    st["bound"] = bound
    return st


def _stats_coef(p, nc, st, grp):
    """reciprocal (DVE) + s + the s-scaled coefficient chain. For GPSIMD
    groups this is emitted in-loop AFTER the bound chain has long completed,
    so the lone DVE reciprocal never stalls the copy queue."""
    A = mybir.AluOpType
    gps = st["gps"]
    eng = st["eng"]
    sv = st["sv"]
    a2, b2, ab, bound = st["a2"], st["b2"], st["ab"], st["bound"]
    inv, s = sv("inv"), sv("s")
    nc.vector.reciprocal(inv[:], bound[:])
    if gps:
        eng.tensor_mul(s[:], inv[:], p.c126[:])
    else:
        eng.tensor_scalar(s[:], inv[:], 126.0, None, A.mult)

    # ---- coefficient chain (c, diag helpers, s-scaled R coefficients) ----
    c, m3c, mc, mcab = sv("c"), sv("m3c"), sv("mc"), sv("mcab")
    eng.tensor_mul(c[:], st["dot"][:], ab[:])
    if gps:
        eng.tensor_mul(m3c[:], c[:], p.cm3[:])
        eng.tensor_mul(mc[:], c[:], p.cm1[:])
    else:
        eng.tensor_scalar(m3c[:], c[:], -3.0, None, A.mult)
        eng.tensor_scalar(mc[:], c[:], -1.0, None, A.mult)
    eng.tensor_mul(mcab[:], mc[:], ab[:])
    A3B, AB3, A4, B4, A2B2 = (sv("A3B"), sv("AB3"), sv("A4"), sv("B4"),
                              sv("A2B2"))
    eng.tensor_mul(A3B[:], a2[:], ab[:])
    eng.tensor_mul(AB3[:], b2[:], ab[:])
    eng.tensor_mul(A4[:], a2[:], a2[:])
    eng.tensor_mul(B4[:], b2[:], b2[:])
    eng.tensor_mul(A2B2[:], ab[:], ab[:])
    cs, m3cs, mcs = sv("cs"), sv("m3cs"), sv("mcs")
    eng.tensor_mul(cs[:], c[:], s[:])
    if gps:
        eng.tensor_mul(m3cs[:], cs[:], p.cm3[:])
        eng.tensor_mul(mcs[:], cs[:], p.cm1[:])
    else:
        eng.tensor_scalar(m3cs[:], cs[:], -3.0, None, A.mult)
        eng.tensor_scalar(mcs[:], cs[:], -1.0, None, A.mult)
    alv, bev, gav, dev, epv = (sv("alv"), sv("bev"), sv("gav"), sv("dev"),
                               sv("epv"))
    eng.tensor_mul(alv[:], m3cs[:], A4[:])
    eng.tensor_mul(bev[:], A3B[:], s[:])
    eng.tensor_mul(gav[:], mcs[:], A2B2[:])
    eng.tensor_mul(dev[:], AB3[:], s[:])
    eng.tensor_mul(epv[:], m3cs[:], B4[:])
    st.update(c=c, m3c=m3c, mcab=mcab,
              alv=alv, bev=bev, gav=gav, dev=dev, epv=epv)
    return st


def _stats_phase1(p, nc, st, grp):
    """R rows in fp16, s pre-folded, packed in ONE tile:
    Rall = [R0 | R1] = [A | B | Dd  |  B | C | E] with
      A = alv*z1 + bev*z2   B = bev*z1   Dd = dev*z2
      C = gav*z1 + dev*z2   E = dev*z1 + epv*z2
    """
    A = mybir.AluOpType
    gps = st["gps"]
    eng = st["eng"]
    z1g, z2g, wt = st["z1g"], st["z2g"], st["wt"]
    Rall = p.rall.tile([P, 2 * KD], f16, tag="Rall", name=f"Rall_{grp}")

    def ts(out, in0, svt):
        if gps:
            eng.tensor_mul(out, in0, svt[:].broadcast_to([P, in0.shape[-1]]))
        else:
            eng.tensor_scalar(out, in0, svt[:], None, A.mult)

    def stt(out, in0, svt, in1, tag):
        if gps:
            w = in0.shape[-1]
            tmp = wt(tag)
            eng.tensor_mul(tmp[:, 0:w], in0,
                           svt[:].broadcast_to([P, w]))
            eng.tensor_add(out, tmp[:, 0:w], in1)
        else:
            eng.scalar_tensor_tensor(out, in0, svt[:], in1, A.mult, A.add)

    alv, bev, gav, dev, epv = (st["alv"], st["bev"], st["gav"], st["dev"],
                               st["epv"])
    t0 = wt("t0")
    ts(t0[:], z2g[:], bev)
    stt(Rall[:, 0:D], z1g[:], alv, t0[:], "p1a")           # A
    ts(Rall[:, D:2 * D], z1g[:], bev)                      # B
    ts(Rall[:, 2 * D:3 * D], z2g[:], dev)                  # Dd
    ts(Rall[:, 3 * D:4 * D], z1g[:], bev)                  # B (R1 copy)
    t1 = wt("t1")
    ts(t1[:], z2g[:], dev)
    stt(Rall[:, 4 * D:5 * D], z1g[:], gav, t1[:], "p1b")   # C
    t2 = wt("t2")
    ts(t2[:], z2g[:], epv)
    stt(Rall[:, 5 * D:6 * D], z1g[:], dev, t2[:], "p1c")   # E
    st["Rall"] = Rall


def _stats_phase2(p, nc, st, grp, diag_hw):
    """Final diagonal values, batch-major [128b, 3*128i]; DMA'd out as f32.
    ALWAYS on GPSIMD (latency-irrelevant; keeps DVE/ACT for copies).
    Host overwrites out[k, b, i, i] with these.
    """
    eng = nc.gpsimd
    wt = st["wt"]

    def sv(tag):
        return p.stat.tile([P, 1], f32, tag=tag, name=f"sv_{tag}_{grp}")

    v1z, v2z, wz = st["v1z"], st["v2z"], st["wz"]
    a2, b2, ab, c, m3c = st["a2"], st["b2"], st["ab"], st["c"], st["m3c"]
    dall = p.dpool.tile([P, KD], f32, tag="dall", name=f"dall_{grp}")

    def bc(svt):
        return svt[:].broadcast_to([P, D])

    twoabw = wt("twoabw")
    ab2 = sv("ab2")
    eng.tensor_add(ab2[:], ab[:], ab[:])
    eng.tensor_mul(twoabw[:], wz[:], bc(ab2))
    # d11 = a2*(c + 2ab*wz + m3c*a2*v1z)
    u1, u2, u2c = wt("u1"), wt("u2"), wt("u2c")
    pa = sv("pa")
    eng.tensor_mul(pa[:], a2[:], m3c[:])
    eng.tensor_mul(u1[:], v1z[:], bc(pa))
    eng.tensor_add(u2[:], u1[:], twoabw[:])
    eng.tensor_add(u2c[:], u2[:], bc(c))
    eng.tensor_mul(dall[:, 0:D], u2c[:], bc(a2))
    # d12 = ab*(a2*v1z + b2*v2z + mcab*wz - 1)
    w1, w2, w2b, w3, w3b, w4 = (wt("w1"), wt("w2"), wt("w2b"), wt("w3"),
                                wt("w3b"), wt("w4"))
    eng.tensor_mul(w1[:], v1z[:], bc(a2))
    eng.tensor_mul(w2[:], v2z[:], bc(b2))
    eng.tensor_add(w2b[:], w2[:], w1[:])
    eng.tensor_mul(w3[:], wz[:], bc(st["mcab"]))
    eng.tensor_add(w3b[:], w3[:], w2b[:])
    eng.tensor_mul(w4[:], w3b[:], bc(ab))
    eng.tensor_sub(dall[:, D:2 * D], w4[:], bc(ab))
    # d22 = b2*(c + 2ab*wz + m3c*b2*v2z)
    u3, u4, u4c = wt("u3"), wt("u4"), wt("u4c")
    pb = sv("pb")
    eng.tensor_mul(pb[:], b2[:], m3c[:])
    eng.tensor_mul(u3[:], v2z[:], bc(pb))
    eng.tensor_add(u4[:], u3[:], twoabw[:])
    eng.tensor_add(u4c[:], u4[:], bc(c))
    eng.tensor_mul(dall[:, 2 * D:3 * D], u4c[:], bc(b2))
    nc.gpsimd.dma_start(diag_hw[grp], dall[:])
    # dequant scale (bound) to DRAM; deferred here so it never blocks the
    # GPSIMD queue head during the ramp
    nc.gpsimd.dma_start(p.scl_hw[grp], st["bound"][:])


def _emit_chunk(p, nc, ZI, RI, out_hw, grp, ch, qctr):
    """G elements (matmul + quad PSUM->int8 copy) + output DMA."""
    e0 = grp * P + ch * G     # global element base for this chunk
    ci = grp * NCH + ch
    GS = 8 if ci in (0, GROUPS * NCH - 1) else 16
    for sub in range(G // GS):
        STG = p.stage.tile([P, GS * KD], i8, tag="STG",
                           name=f"STG_{grp}_{ch}_{sub}")
        for t in range(GS // QB):
            pt = p.mmp.tile([P, QB * 512], f32, tag="pt",
                            name=f"pt_{grp}_{ch}_{sub}_{t}")
            for slot in range(QB):
                s = sub * GS + t * QB + slot   # local element in chunk
                half = s % 2                   # PE quadrant alternation
                ff = ch * (G // 2) + s // 2    # column within group tile
                pp = 32 * half
                lhsT = ZI[pp:pp + 2, ff * D:(ff + 1) * D]
                rhs = RI[pp:pp + 2, ff * KD:(ff + 1) * KD]
                nc.tensor.matmul(pt[:, slot * 512:slot * 512 + KD],
                                 lhsT, rhs, start=True, stop=True)
            src = pt[:].rearrange("p (q c) -> p q c", c=512)[:, :, 0:KD]
            dst = STG[:, t * QB * KD:(t + 1) * QB * KD].rearrange(
                "p (q c) -> p q c", c=KD)
            if _copy_engine(qctr[0]):
                nc.scalar.copy(dst, src)
            else:
                nc.vector.tensor_copy(dst, src)
            qctr[0] += 1
        es = e0 + sub * GS
        nc.sync.dma_start(out_hw[:, es * KD:(es + GS) * KD], STG[:])


def _build_body(ctx, tc, z1, z2, out_hw, diag_hw, scl_hw):
    nc = tc.nc
    p = _make_pools(ctx, tc)
    p.scl_hw = scl_hw
    _make_consts(p, nc)

    qctr = [0]
    zs = _load_all_z(p, nc, z1, z2)
    # z fp16 casts + lhsT gathers have NO stats dependency -> fire at t=0
    zh = {0: _cast_zh(p, nc, zs, 0), 1: _cast_zh(p, nc, zs, 1)}
    gzi = {0: _emit_zi(p, nc, zh[0], 0), 1: _emit_zi(p, nc, zh[1], 1)}
    # group 0-1 stats on DVE during the ramp window
    # group 0 chain on DVE: the ramp-critical path
    sts = {0: _stats_norms_tt(p, nc, zs, 0)}
    _stats_norms_fin(p, nc, sts[0], 0)
    _stats_scale(p, nc, sts[0], 0)
    _stats_coef(p, nc, sts[0], 0)
    _stats_phase1(p, nc, sts[0], 0)
    gri = {0: _emit_ri(p, nc, sts[0]["Rall"], 0)}
    # groups 2-3 norms upfront (GPSIMD TTs + DVE reduces + ACT sqrts): their
    # sqrt hops run on ACT BEFORE the copy stream starts, never stalling it
    sts[2] = _stats_norms_tt(p, nc, zs, 2)
    sts[3] = _stats_norms_tt(p, nc, zs, 3)
    _stats_norms_fin(p, nc, sts[2], 2)
    _stats_norms_fin(p, nc, sts[3], 3)
    # group 1 chain on DVE
    sts[1] = _stats_norms_tt(p, nc, zs, 1)
    _stats_norms_fin(p, nc, sts[1], 1)
    _stats_scale(p, nc, sts[1], 1)
    _stats_coef(p, nc, sts[1], 1)
    _stats_phase1(p, nc, sts[1], 1)
    # groups 2-3 bound chains: pure GPSIMD, all inputs ready -> run in the
    # ramp window, emitted BEFORE RI1 whose Rall1-wait blocks the GPSIMD
    # queue head until ~32us
    _stats_scale(p, nc, sts[2], 2)
    _stats_scale(p, nc, sts[3], 3)
    gri[1] = _emit_ri(p, nc, sts[1]["Rall"], 1)
    # diag work entirely on GPSIMD, never latency-critical
    _stats_phase2(p, nc, sts[0], 0, diag_hw)
    _stats_phase2(p, nc, sts[1], 1, diag_hw)
    zh[2] = _cast_zh(p, nc, zs, 2)
    zh[3] = _cast_zh(p, nc, zs, 3)
    TOT = GROUPS * NCH
    for ci in range(TOT):
        grp, ch = divmod(ci, NCH)
        if ch == 0 and grp in (1, 2):
            # groups 2-3: gathers 3.5 chunks ahead of first use, emitted as
            # soon as the previous group's chunks (the buffer's last readers)
            # are all emitted, so the GPSIMD queue has slack to deliver them
            gzi[grp + 1] = _emit_zi(p, nc, zh[grp + 1], grp + 1)
            gri[grp + 1] = _emit_ri(p, nc, sts[grp + 1]["Rall"], grp + 1)
        _emit_chunk(p, nc, gzi[grp], gri[grp], out_hw, grp, ch, qctr)
        # remaining stats for group g+2 on GPSIMD (norms already done in the
        # ramp window; only the GPS-local coef/R/diag chains remain)
        if grp + 2 < GROUPS:
            if ch == 1:
                _stats_coef(p, nc, sts[grp + 2], grp + 2)
            elif ch == 2:
                _stats_phase1(p, nc, sts[grp + 2], grp + 2)
            elif ch == 3:
                _stats_phase2(p, nc, sts[grp + 2], grp + 2, diag_hw)


def build_kernel():
    nc = bacc.Bacc("TRN2", target_bir_lowering=False, debug=False)
    z1 = nc.dram_tensor("z1", [B_SH, D], f32, kind="ExternalInput").ap()
    z2 = nc.dram_tensor("z2", [B_SH, D], f32, kind="ExternalInput").ap()
    # packed rank-2 output: [i partition, (b, k, j) free], int8
    out_hw = nc.dram_tensor("out", [P, B_SH * KD], i8,
                            kind="ExternalOutput").ap()
    scl_hw = nc.dram_tensor("scl", [GROUPS, P, 1], f32,
                            kind="ExternalOutput").ap()
    # final diagonals: [group, b-partition (interleaved), (k, i) free], f32
    diag_hw = nc.dram_tensor("diag", [GROUPS, P, KD], f32,
                             kind="ExternalOutput").ap()
    with tile.TileContext(nc) as tc:
        with ExitStack() as ctx:
            _build_body(ctx, tc, z1, z2, out_hw, diag_hw, scl_hw)
    nc.compile()
    return nc


_NC_CACHE = None


def _get_nc():
    global _NC_CACHE
    if _NC_CACHE is None:
        _NC_CACHE = build_kernel()
    return _NC_CACHE


def _assemble(out_c, diag_c, scl_c, dst):
    """Unpack one core's HW tensors into dst [3, B_SH, D, D] f32."""
    # out_c [128 i, B_SH*384] int8 -> [i, b, k, j] -> [k, b, i, j]; dequant
    # with the per-element scale bound/126 (scl rows are partition-ordered)
    h = out_c.reshape(P, B_SH, 3, D).transpose(2, 1, 0, 3)
    scale = (scl_c.reshape(GROUPS, P)[:, PART_OF_ELEM].reshape(B_SH)
             / np.float32(126.0))
    np.multiply(h, scale[None, :, None, None], out=dst, dtype=np.float32)
    # diag_c [GROUPS, 128 p, 3*128] f32; partition p holds group element
    # E_OF_P... i.e. ordered element e sits at partition P_OF_E[e]
    dv = diag_c[:, PART_OF_ELEM, :].reshape(GROUPS, P, 3, D).transpose(
        2, 0, 1, 3).reshape(3, B_SH, D)
    dst.reshape(3, B_SH, D * D)[:, :, ::D + 1] = dv


def kernel(z1, z2):
    nc = _get_nc()
    z1 = np.ascontiguousarray(np.asarray(z1, dtype=np.float32))
    z2 = np.ascontiguousarray(np.asarray(z2, dtype=np.float32))
    in_maps = [
        {"z1": z1[c * B_SH:(c + 1) * B_SH], "z2": z2[c * B_SH:(c + 1) * B_SH]}
        for c in range(N_CORES)
    ]
    res = run_bass_kernel_spmd(nc, in_maps, core_ids=list(range(N_CORES)))
    full = np.empty((3, B_FULL, D, D), dtype=np.float32)
    for c in range(N_CORES):
        _assemble(res.results[c]["out"], res.results[c]["diag"],
                  res.results[c]["scl"], full[:, c * B_SH:(c + 1) * B_SH])
    return full


# revision 46
# speedup vs baseline: 1.0438x; 1.0319x over previous
"""Trainium2 Bass kernel for ArccosHessianCalculator (int8-packed output).

Math: for each batch element b (z1, z2 are [B, D] with D = 128):
  a = 1/|z1|, bb = 1/|z2|, c = cos = <z1u, z2u>
  Each Hessian block H_k is rank-2 plus a diagonal term:
      H_k(b) = z1 * r0_k(b)^T + z2 * r1_k(b)^T + diag
  with the normalization / cosine factors folded into r0/r1:
      k=0 (H11): r0 = -3c*a^4*z1 + a^3 b*z2          r1 = a^3 b*z1
      k=1 (H12): r0 = a^3 b*z1                        r1 = -c*a^2 b^2*z1 + a b^3*z2
      k=2 (H22): r0 = a b^3*z2                        r1 = a b^3*z1 - 3c*b^4*z2

Device strategy (per core, batch shard of 512):
  - TensorE: one K=2 matmul per element, lhsT = [z1(b); z2(b)] ([2,128] fp16),
    rhs = [r0 | r1] blocks ([2,384] fp16) -> PSUM f32 [128, 384]. Elements
    alternate PE quadrant rows ({0,1} / {32,33}) so LDWEIGHTS for element
    e+1 overlaps the stream of element e.
  - INT8 output: a guaranteed per-element bound on |H_rank2| (triangle
    inequality with |c|<=1 and AM-GM, ~16 [128,1] ops) gives s = 126/bound.
    s is folded ENTIRELY into the R coefficients, so the z fp16 casts have
    NO stats dependency (lhsT gathers fire at t=0) and PSUM holds s*H; the
    PSUM->SBUF copy is a plain int8 cast. The rank-2 part lands in DRAM
    packed as out_hw [128 i, B_SH*384 (b,k,j)] int8 -- 4x less HBM traffic
    than f32.
  - Final diagonals are computed in closed form per group ([128 b, 3*128 i])
    into a small f32 side tensor and spliced on the HOST in f32 (they
    dominate the Hessian's magnitude, so they carry no quantization error).
  - PSUM: 2-bank super-tiles x4 buffers; each drained by ONE strided copy
    (2 elements) split 5:6 between DVE and ACT (weighted by their measured
    copy rates; PSUM reads are f32 at ~1 elem/cycle/partition).
  - Stats: group 0-1 chains on DVE (short ramp); groups 2-3 and ALL diag
    (phase2) work on GPSIMD, which is otherwise idle, so recurring stats
    never steal copy throughput. Free-axis reduces / reciprocal stay on DVE
    (GPSIMD cannot do them), sqrt on ACT. All z tiles prefetch at t=0.
  - Batch rows are loaded interleaved (even elements -> partitions 0..63,
    odd -> 64..127) so gathers are plain partition slices; z and R live in
    combined tiles (zh [128,256], Rall [128,768]) so each gather half is a
    single 2-partition DMA (4 dispatches per group instead of 8).
  - Host: reshape/transpose view + per-element dequant multiply + f32
    diagonal stride-trick splice.
"""

import numpy as np
from contextlib import ExitStack

import concourse.bass as bass
import concourse.tile as tile
from concourse import bacc, mybir
from concourse.bass_utils import run_bass_kernel_spmd

N_CORES = 8
B_FULL = 4096
D = 128
B_SH = B_FULL // N_CORES  # 512 batch elements per core
P = 128                   # SBUF partitions
KD = 3 * D                # 384: three H blocks side by side
G = 32                    # elements per chunk
GROUPS = B_SH // P        # 4 stats groups of 128 elements
NCH = P // G              # 4 chunks per group
QB = 2                    # elements per PSUM super-tile (2 banks)
OC = 320                  # output cols per element (256 main + 64 packed corners)
HF = P // 2               # 64 elements per interleaved half

f32 = mybir.dt.float32
f16 = mybir.dt.float16
i8 = mybir.dt.int8

# interleaved element order within a group: partition p holds group element
# 2p (p < 64) or 2(p-64)+1 (p >= 64)
ELEM_OF_PART = np.concatenate([np.arange(0, P, 2), np.arange(1, P, 2)])
PART_OF_ELEM = np.argsort(ELEM_OF_PART)

# copy-engine schedule: 6 ACT : 5 DVE interleaved (ACT is ~1.18x faster at
# PSUM->SBUF f32 copies: (172+FD)/1.2GHz vs (120+FD)/0.96GHz).
# The first 18 super-tiles run 2:1 ACT-heavy: DVE is still draining the
# group-1 stats chain during the ramp-out.
COPY_PAT = [1, 0, 1, 0, 1, 0, 1, 0, 1, 0, 1]  # 1 = ACT


def _copy_engine(q):
    if q < 18:
        return 1 if q % 3 != 2 else 0
    return COPY_PAT[q % len(COPY_PAT)]


class _Pools:
    pass


def _make_pools(ctx, tc):
    p = _Pools()
    p.const = ctx.enter_context(tc.tile_pool(name="const", bufs=1))
    p.zg = ctx.enter_context(tc.tile_pool(name="zg", bufs=1))
    p.zh = ctx.enter_context(tc.tile_pool(name="zh", bufs=4))
    p.work = ctx.enter_context(tc.tile_pool(name="work", bufs=2))
    p.nrm = ctx.enter_context(tc.tile_pool(name="nrm", bufs=1))
    p.stat = ctx.enter_context(tc.tile_pool(name="stat", bufs=3))
    p.rall = ctx.enter_context(tc.tile_pool(name="rall", bufs=2))
    p.dpool = ctx.enter_context(tc.tile_pool(name="dpool", bufs=2))
    p.zi = ctx.enter_context(tc.tile_pool(name="zi", bufs=2))
    p.ri = ctx.enter_context(tc.tile_pool(name="ri", bufs=2))
    p.stage = ctx.enter_context(tc.tile_pool(name="stage", bufs=5))
    p.mmp = ctx.enter_context(tc.tile_pool(name="mmp", bufs=4, space="PSUM"))
    return p


def _make_consts(p, nc):
    """[128,1] constant tiles for GPSIMD (Pool has no scalar immediates)."""
    for tag, val in (("c3", 3.0), ("c075", 0.75), ("c126", 126.0),
                     ("cm1", -1.0), ("cm3", -3.0)):
        t = p.const.tile([P, 1], f32, tag=tag, name=tag)
        nc.vector.memset(t[:], val)
        setattr(p, tag, t)


def _load_all_z(p, nc, z1, z2):
    """Prefetch every group's z tiles at t=0 (tiny: 512B/partition each)."""
    zs = {}
    for grp in range(GROUPS):
        b0 = grp * P
        ldma = nc.sync if grp == 0 else nc.gpsimd
        # interleaved row order: partition p <- batch row 2p / 2(p-64)+1
        z1g = p.zg.tile([P, D], f32, tag=f"z1g{grp}", name=f"z1g_{grp}")
        ldma.dma_start(z1g[:], z1[b0:b0 + P, :].rearrange(
            "(f two) d -> two f d", two=2))
        z2g = p.zg.tile([P, D], f32, tag=f"z2g{grp}", name=f"z2g_{grp}")
        ldma.dma_start(z2g[:], z2[b0:b0 + P, :].rearrange(
            "(f two) d -> two f d", two=2))
        zs[grp] = (z1g, z2g)
    return zs


def _cast_zh(p, nc, zs, grp):
    """Plain fp16 casts of z1/z2 into ONE tile (no stats dependency)."""
    z1g, z2g = zs[grp]
    zh = p.zh.tile([P, 2 * D], f16, tag=f"zh{grp}", name=f"zh_{grp}")
    nc.scalar.copy(zh[:, 0:D], z1g[:])
    nc.scalar.copy(zh[:, D:2 * D], z2g[:])
    return zh


def _emit_zi(p, nc, zh, grp):
    """lhsT gather: one 2-partition DMA per interleaved half."""
    ZI = p.zi.tile([P, HF * D], f16, tag="ZI", name=f"ZI_{grp}")
    dmae = nc.sync if grp == 0 else nc.gpsimd
    for half in range(2):
        hb, pp = HF * half, 32 * half
        dmae.dma_start(ZI[pp:pp + 1, :], zh[hb:hb + HF, 0:D])
        dmae.dma_start(ZI[pp + 1:pp + 2, :], zh[hb:hb + HF, D:2 * D])
    return ZI


def _emit_ri(p, nc, Rall, grp):
    """rhs gather. Group 0 is split into prefix waves so chunk 0's first
    matmuls only wait on a tiny first DMA (Tile tracks write ranges), the
    rest follows while they run. Later groups: one DMA per row."""
    RI = p.ri.tile([P, HF * KD], f16, tag="RI", name=f"RI_{grp}")
    dmae = nc.sync if grp == 0 else nc.gpsimd
    waves = [(0, 12), (12, 24), (24, 44), (44, HF)] if grp == 0 else [(0, HF)]
    for lo, hi in waves:
        for half in range(2):
            hb, pp = HF * half, 32 * half
            dmae.dma_start(RI[pp:pp + 1, lo * KD:hi * KD],
                           Rall[hb + lo:hb + hi, 0:KD])
            dmae.dma_start(RI[pp + 1:pp + 2, lo * KD:hi * KD],
                           Rall[hb + lo:hb + hi, KD:2 * KD])
    return RI


def _stats_norms_tt(p, nc, zs, grp):
    """Elementwise squares/products (on the group's stats engine)."""
    gps = grp >= 2
    eng = nc.gpsimd if gps else nc.vector
    z1g, z2g = zs[grp]

    def wt(tag):
        return p.work.tile([P, D], f32, tag=tag, name=f"w_{tag}_{grp}")

    def sv(tag):
        return p.stat.tile([P, 1], f32, tag=tag, name=f"sv_{tag}_{grp}")

    def nv(tag, wide=False):
        return p.nrm.tile([P, D if wide else 1], f32, tag=f"{tag}{grp}",
                          name=f"n_{tag}_{grp}")

    st = {"eng": eng, "gps": gps, "wt": wt, "sv": sv, "nv": nv,
          "z1g": z1g, "z2g": z2g}
    v1z, v2z, wz = nv("v1z", True), nv("v2z", True), nv("wz", True)
    eng.tensor_mul(v1z[:], z1g[:], z1g[:])
    eng.tensor_mul(v2z[:], z2g[:], z2g[:])
    eng.tensor_mul(wz[:], z1g[:], z2g[:])
    st.update(v1z=v1z, v2z=v2z, wz=wz)
    return st


def _stats_norms_fin(p, nc, st, grp):
    """Reduces/reciprocals (DVE-only) + ab2t on the stats engine + ACT sqrt.
    Emitted upfront for ALL groups: the sqrt hops land on ACT before the
    copy stream starts, so they never stall copies mid-kernel."""
    eng = st["eng"]
    nv = st["nv"]
    v1z, v2z, wz = st["v1z"], st["v2z"], st["wz"]
    s1, s2, dot = nv("s1"), nv("s2"), nv("dot")
    nc.vector.reduce_sum(s1[:], v1z[:], axis=mybir.AxisListType.X)
    nc.vector.reduce_sum(s2[:], v2z[:], axis=mybir.AxisListType.X)
    nc.vector.reduce_sum(dot[:], wz[:], axis=mybir.AxisListType.X)
    mz1, mz2 = nv("mz1"), nv("mz2")
    nc.vector.reduce_max(mz1[:], st["z1g"][:], axis=mybir.AxisListType.X,
                         apply_absolute_value=True)
    nc.vector.reduce_max(mz2[:], st["z2g"][:], axis=mybir.AxisListType.X,
                         apply_absolute_value=True)
    a2, b2 = nv("a2"), nv("b2")
    nc.vector.reciprocal(a2[:], s1[:])
    nc.vector.reciprocal(b2[:], s2[:])
    ab2t, ab = nv("ab2t"), nv("ab")
    eng.tensor_mul(ab2t[:], a2[:], b2[:])
    nc.scalar.sqrt(ab[:], ab2t[:])
    st.update(s1=s1, s2=s2, dot=dot, mz1=mz1, mz2=mz2, a2=a2, b2=b2,
              ab2t=ab2t, ab=ab)
    return st


def _stats_scale(p, nc, st, grp):
    """int8 bound -> s = 126/bound, plus the s-scaled coefficient set.

    Bound (valid upper bound on max_k |H_k_rank2[i,j]|, using |c|<=1 and
    mu*mv <= (mu^2+mv^2)/2, ab <= (a2+b2)/2):
      pp = (mz1*a)^2, qq = (mz2*b)^2, pq = pp+qq
      bound = a2*(3pp+pq) + 0.75*(a2+b2)*pq + b2*(3qq+pq)
    """
    A = mybir.AluOpType
    gps = st["gps"]
    eng = st["eng"]
    sv = st["sv"]
    mz1, mz2, a2, b2, ab = st["mz1"], st["mz2"], st["a2"], st["b2"], st["ab"]
    mz1s, mz2s, pp_, qq, pq = (sv("mz1s"), sv("mz2s"), sv("pp"), sv("qq"),
                               sv("pq"))
    eng.tensor_mul(mz1s[:], mz1[:], mz1[:])
    eng.tensor_mul(mz2s[:], mz2[:], mz2[:])
    eng.tensor_mul(pp_[:], mz1s[:], a2[:])
    eng.tensor_mul(qq[:], mz2s[:], b2[:])
    eng.tensor_add(pq[:], pp_[:], qq[:])
    t11, s11, b11 = sv("t11"), sv("s11"), sv("b11")
    t22, s22, b22 = sv("t22"), sv("s22"), sv("b22")
    hh, s12, b12 = sv("hh"), sv("s12"), sv("b12")
    if gps:
        eng.tensor_mul(t11[:], pp_[:], p.c3[:])
        eng.tensor_mul(t22[:], qq[:], p.c3[:])
    else:
        eng.tensor_scalar(t11[:], pp_[:], 3.0, None, A.mult)
        eng.tensor_scalar(t22[:], qq[:], 3.0, None, A.mult)
    eng.tensor_add(s11[:], t11[:], pq[:])
    eng.tensor_mul(b11[:], s11[:], a2[:])
    eng.tensor_add(s22[:], t22[:], pq[:])
    eng.tensor_mul(b22[:], s22[:], b2[:])
    eng.tensor_add(hh[:], a2[:], b2[:])
    eng.tensor_mul(s12[:], pq[:], hh[:])
    if gps:
        eng.tensor_mul(b12[:], s12[:], p.c075[:])
    else:
        eng.tensor_scalar(b12[:], s12[:], 0.75, None, A.mult)
    bs, inv, s = sv("bs"), sv("inv"), sv("s")
    bound = st["nv"]("bound")
    eng.tensor_add(bs[:], b11[:], b12[:])
    eng.tensor_add(bound[:], bs[:], b22[:])
    st["bound"] = bound
    return st


def _stats_coef(p, nc, st, grp):
    """reciprocal (DVE) + s + the s-scaled coefficient chain. For GPSIMD
    groups this is emitted in-loop AFTER the bound chain has long completed,
    so the lone DVE reciprocal never stalls the copy queue."""
    A = mybir.AluOpType
    gps = st["gps"]
    eng = st["eng"]
    sv = st["sv"]
    a2, b2, ab, bound = st["a2"], st["b2"], st["ab"], st["bound"]
    inv, s = sv("inv"), sv("s")
    nc.vector.reciprocal(inv[:], bound[:])
    if gps:
        eng.tensor_mul(s[:], inv[:], p.c126[:])
    else:
        eng.tensor_scalar(s[:], inv[:], 126.0, None, A.mult)

    # ---- coefficient chain (c, diag helpers, s-scaled R coefficients) ----
    c, m3c, mc, mcab = sv("c"), sv("m3c"), sv("mc"), sv("mcab")
    eng.tensor_mul(c[:], st["dot"][:], ab[:])
    if gps:
        eng.tensor_mul(m3c[:], c[:], p.cm3[:])
        eng.tensor_mul(mc[:], c[:], p.cm1[:])
    else:
        eng.tensor_scalar(m3c[:], c[:], -3.0, None, A.mult)
        eng.tensor_scalar(mc[:], c[:], -1.0, None, A.mult)
    eng.tensor_mul(mcab[:], mc[:], ab[:])
    A3B, AB3, A4, B4, A2B2 = (sv("A3B"), sv("AB3"), sv("A4"), sv("B4"),
                              sv("A2B2"))
    eng.tensor_mul(A3B[:], a2[:], ab[:])
    eng.tensor_mul(AB3[:], b2[:], ab[:])
    eng.tensor_mul(A4[:], a2[:], a2[:])
    eng.tensor_mul(B4[:], b2[:], b2[:])
    eng.tensor_mul(A2B2[:], ab[:], ab[:])
    cs, m3cs, mcs = sv("cs"), sv("m3cs"), sv("mcs")
    eng.tensor_mul(cs[:], c[:], s[:])
    if gps:
        eng.tensor_mul(m3cs[:], cs[:], p.cm3[:])
        eng.tensor_mul(mcs[:], cs[:], p.cm1[:])
    else:
        eng.tensor_scalar(m3cs[:], cs[:], -3.0, None, A.mult)
        eng.tensor_scalar(mcs[:], cs[:], -1.0, None, A.mult)
    alv, bev, gav, dev, epv = (sv("alv"), sv("bev"), sv("gav"), sv("dev"),
                               sv("epv"))
    eng.tensor_mul(alv[:], m3cs[:], A4[:])
    eng.tensor_mul(bev[:], A3B[:], s[:])
    eng.tensor_mul(gav[:], mcs[:], A2B2[:])
    eng.tensor_mul(dev[:], AB3[:], s[:])
    eng.tensor_mul(epv[:], m3cs[:], B4[:])
    st.update(c=c, m3c=m3c, mcab=mcab,
              alv=alv, bev=bev, gav=gav, dev=dev, epv=epv)
    return st


def _stats_phase1(p, nc, st, grp):
    """R rows in fp16, s pre-folded, packed in ONE tile:
    Rall = [R0 | R1] = [A | B | Dd  |  B | C | E] with
      A = alv*z1 + bev*z2   B = bev*z1   Dd = dev*z2
      C = gav*z1 + dev*z2   E = dev*z1 + epv*z2
    """
    A = mybir.AluOpType
    gps = st["gps"]
    eng = st["eng"]
    z1g, z2g, wt = st["z1g"], st["z2g"], st["wt"]
    Rall = p.rall.tile([P, 2 * KD], f16, tag="Rall", name=f"Rall_{grp}")

    def ts(out, in0, svt):
        if gps:
            eng.tensor_mul(out, in0, svt[:].broadcast_to([P, in0.shape[-1]]))
        else:
            eng.tensor_scalar(out, in0, svt[:], None, A.mult)

    def stt(out, in0, svt, in1, tag):
        if gps:
            w = in0.shape[-1]
            tmp = wt(tag)
            eng.tensor_mul(tmp[:, 0:w], in0,
                           svt[:].broadcast_to([P, w]))
            eng.tensor_add(out, tmp[:, 0:w], in1)
        else:
            eng.scalar_tensor_tensor(out, in0, svt[:], in1, A.mult, A.add)

    alv, bev, gav, dev, epv = (st["alv"], st["bev"], st["gav"], st["dev"],
                               st["epv"])
    t0 = wt("t0")
    ts(t0[:], z2g[:], bev)
    stt(Rall[:, 0:D], z1g[:], alv, t0[:], "p1a")           # A
    ts(Rall[:, D:2 * D], z1g[:], bev)                      # B
    ts(Rall[:, 2 * D:3 * D], z2g[:], dev)                  # Dd
    ts(Rall[:, 3 * D:4 * D], z1g[:], bev)                  # B (R1 copy)
    t1 = wt("t1")
    ts(t1[:], z2g[:], dev)
    stt(Rall[:, 4 * D:5 * D], z1g[:], gav, t1[:], "p1b")   # C
    t2 = wt("t2")
    ts(t2[:], z2g[:], epv)
    stt(Rall[:, 5 * D:6 * D], z1g[:], dev, t2[:], "p1c")   # E
    st["Rall"] = Rall


def _stats_phase2(p, nc, st, grp, diag_hw):
    """Final diagonal values, batch-major [128b, 3*128i]; DMA'd out as f32.
    ALWAYS on GPSIMD (latency-irrelevant; keeps DVE/ACT for copies).
    Host overwrites out[k, b, i, i] with these.
    """
    eng = nc.gpsimd
    wt = st["wt"]

    def sv(tag):
        return p.stat.tile([P, 1], f32, tag=tag, name=f"sv_{tag}_{grp}")

    v1z, v2z, wz = st["v1z"], st["v2z"], st["wz"]
    a2, b2, ab, c, m3c = st["a2"], st["b2"], st["ab"], st["c"], st["m3c"]
    dall = p.dpool.tile([P, KD], f32, tag="dall", name=f"dall_{grp}")

    def bc(svt):
        return svt[:].broadcast_to([P, D])

    twoabw = wt("twoabw")
    ab2 = sv("ab2")
    eng.tensor_add(ab2[:], ab[:], ab[:])
    eng.tensor_mul(twoabw[:], wz[:], bc(ab2))
    # d11 = a2*(c + 2ab*wz + m3c*a2*v1z)
    u1, u2, u2c = wt("u1"), wt("u2"), wt("u2c")
    pa = sv("pa")
    eng.tensor_mul(pa[:], a2[:], m3c[:])
    eng.tensor_mul(u1[:], v1z[:], bc(pa))
    eng.tensor_add(u2[:], u1[:], twoabw[:])
    eng.tensor_add(u2c[:], u2[:], bc(c))
    eng.tensor_mul(dall[:, 0:D], u2c[:], bc(a2))
    # d12 = ab*(a2*v1z + b2*v2z + mcab*wz - 1)
    w1, w2, w2b, w3, w3b, w4 = (wt("w1"), wt("w2"), wt("w2b"), wt("w3"),
                                wt("w3b"), wt("w4"))
    eng.tensor_mul(w1[:], v1z[:], bc(a2))
    eng.tensor_mul(w2[:], v2z[:], bc(b2))
    eng.tensor_add(w2b[:], w2[:], w1[:])
    eng.tensor_mul(w3[:], wz[:], bc(st["mcab"]))
    eng.tensor_add(w3b[:], w3[:], w2b[:])
    eng.tensor_mul(w4[:], w3b[:], bc(ab))
    eng.tensor_sub(dall[:, D:2 * D], w4[:], bc(ab))
    # d22 = b2*(c + 2ab*wz + m3c*b2*v2z)
    u3, u4, u4c = wt("u3"), wt("u4"), wt("u4c")
    pb = sv("pb")
    eng.tensor_mul(pb[:], b2[:], m3c[:])
    eng.tensor_mul(u3[:], v2z[:], bc(pb))
    eng.tensor_add(u4[:], u3[:], twoabw[:])
    eng.tensor_add(u4c[:], u4[:], bc(c))
    eng.tensor_mul(dall[:, 2 * D:3 * D], u4c[:], bc(b2))
    nc.gpsimd.dma_start(diag_hw[grp], dall[:])
    # dequant scale (bound) to DRAM; deferred here so it never blocks the
    # GPSIMD queue head during the ramp
    nc.gpsimd.dma_start(p.scl_hw[grp], st["bound"][:])


def _emit_chunk(p, nc, ZI, RI, out_hw, grp, ch, qctr):
    """G elements (matmul + quad PSUM->int8 copy) + output DMA."""
    e0 = grp * P + ch * G     # global element base for this chunk
    ci = grp * NCH + ch
    GS = 8 if ci in (0, GROUPS * NCH - 1) else 16
    for sub in range(G // GS):
        STG = p.stage.tile([P, GS * KD], i8, tag="STG",
                           name=f"STG_{grp}_{ch}_{sub}")
        for t in range(GS // QB):
            pt = p.mmp.tile([P, QB * 512], f32, tag="pt",
                            name=f"pt_{grp}_{ch}_{sub}_{t}")
            for slot in range(QB):
                s = sub * GS + t * QB + slot   # local element in chunk
                half = s % 2                   # PE quadrant alternation
                ff = ch * (G // 2) + s // 2    # column within group tile
                pp = 32 * half
                lhsT = ZI[pp:pp + 2, ff * D:(ff + 1) * D]
                rhs = RI[pp:pp + 2, ff * KD:(ff + 1) * KD]
                nc.tensor.matmul(pt[:, slot * 512:slot * 512 + KD],
                                 lhsT, rhs, start=True, stop=True)
            src = pt[:].rearrange("p (q c) -> p q c", c=512)[:, :, 0:KD]
            dst = STG[:, t * QB * KD:(t + 1) * QB * KD].rearrange(
                "p (q c) -> p q c", c=KD)
            if _copy_engine(qctr[0]):
                nc.scalar.copy(dst, src)
            else:
                nc.vector.tensor_copy(dst, src)
            qctr[0] += 1
        es = e0 + sub * GS
        nc.sync.dma_start(out_hw[:, es * KD:(es + GS) * KD], STG[:])


def _build_body(ctx, tc, z1, z2, out_hw, diag_hw, scl_hw):
    nc = tc.nc
    p = _make_pools(ctx, tc)
    p.scl_hw = scl_hw
    _make_consts(p, nc)

    qctr = [0]
    zs = _load_all_z(p, nc, z1, z2)
    # z fp16 casts + lhsT gathers have NO stats dependency -> fire at t=0
    zh = {0: _cast_zh(p, nc, zs, 0), 1: _cast_zh(p, nc, zs, 1)}
    gzi = {0: _emit_zi(p, nc, zh[0], 0), 1: _emit_zi(p, nc, zh[1], 1)}
    # group 0-1 stats on DVE during the ramp window
    # group 0 chain on DVE: the ramp-critical path
    sts = {0: _stats_norms_tt(p, nc, zs, 0)}
    _stats_norms_fin(p, nc, sts[0], 0)
    _stats_scale(p, nc, sts[0], 0)
    _stats_coef(p, nc, sts[0], 0)
    _stats_phase1(p, nc, sts[0], 0)
    gri = {0: _emit_ri(p, nc, sts[0]["Rall"], 0)}
    # groups 2-3 norms upfront (GPSIMD TTs + DVE reduces + ACT sqrts): their
    # sqrt hops run on ACT BEFORE the copy stream starts, never stalling it
    sts[2] = _stats_norms_tt(p, nc, zs, 2)
    sts[3] = _stats_norms_tt(p, nc, zs, 3)
    _stats_norms_fin(p, nc, sts[2], 2)
    _stats_norms_fin(p, nc, sts[3], 3)
    # group 1 chain on DVE
    sts[1] = _stats_norms_tt(p, nc, zs, 1)
    _stats_norms_fin(p, nc, sts[1], 1)
    _stats_scale(p, nc, sts[1], 1)
    _stats_coef(p, nc, sts[1], 1)
    _stats_phase1(p, nc, sts[1], 1)
    # groups 2-3 bound chains: pure GPSIMD, all inputs ready -> run in the
    # ramp window, emitted BEFORE RI1 whose Rall1-wait blocks the GPSIMD
    # queue head until ~32us
    _stats_scale(p, nc, sts[2], 2)
    _stats_scale(p, nc, sts[3], 3)
    gri[1] = _emit_ri(p, nc, sts[1]["Rall"], 1)
    # diag work entirely on GPSIMD, never latency-critical
    _stats_phase2(p, nc, sts[0], 0, diag_hw)
    _stats_phase2(p, nc, sts[1], 1, diag_hw)
    zh[2] = _cast_zh(p, nc, zs, 2)
    zh[3] = _cast_zh(p, nc, zs, 3)
    TOT = GROUPS * NCH
    for ci in range(TOT):
        grp, ch = divmod(ci, NCH)
        if ch == 0 and grp in (1, 2):
            # groups 2-3: gathers 3.5 chunks ahead of first use, emitted as
            # soon as the previous group's chunks (the buffer's last readers)
            # are all emitted, so the GPSIMD queue has slack to deliver them
            gzi[grp + 1] = _emit_zi(p, nc, zh[grp + 1], grp + 1)
            gri[grp + 1] = _emit_ri(p, nc, sts[grp + 1]["Rall"], grp + 1)
        _emit_chunk(p, nc, gzi[grp], gri[grp], out_hw, grp, ch, qctr)
        # remaining stats for group g+2 on GPSIMD (norms already done in the
        # ramp window; only the GPS-local coef/R/diag chains remain)
        if grp + 2 < GROUPS:
            if ch == 1:
                _stats_coef(p, nc, sts[grp + 2], grp + 2)
            elif ch == 2:
                _stats_phase1(p, nc, sts[grp + 2], grp + 2)
            elif ch == 3:
                _stats_phase2(p, nc, sts[grp + 2], grp + 2, diag_hw)


def build_kernel():
    nc = bacc.Bacc("TRN2", target_bir_lowering=False, debug=False)
    z1 = nc.dram_tensor("z1", [B_SH, D], f32, kind="ExternalInput").ap()
    z2 = nc.dram_tensor("z2", [B_SH, D], f32, kind="ExternalInput").ap()
    # packed rank-2 output: [i partition, (b, k, j) free], int8
    out_hw = nc.dram_tensor("out", [P, B_SH * KD], i8,
                            kind="ExternalOutput").ap()
    scl_hw = nc.dram_tensor("scl", [GROUPS, P, 1], f32,
                            kind="ExternalOutput").ap()
    # final diagonals: [group, b-partition (interleaved), (k, i) free], f32
    diag_hw = nc.dram_tensor("diag", [GROUPS, P, KD], f32,
                             kind="ExternalOutput").ap()
    with tile.TileContext(nc) as tc:
        with ExitStack() as ctx:
            _build_body(ctx, tc, z1, z2, out_hw, diag_hw, scl_hw)
    nc.compile()
    return nc


_NC_CACHE = None


def _get_nc():
    global _NC_CACHE
    if _NC_CACHE is None:
        _NC_CACHE = build_kernel()
    return _NC_CACHE


def _assemble(out_c, diag_c, scl_c, dst):
    """Unpack one core's HW tensors into dst [3, B_SH, D, D] f32."""
    # out_c [128 i, B_SH*384] int8 -> [i, b, k, j] -> [k, b, i, j]; dequant
    # with the per-element scale bound/126 (scl rows are partition-ordered)
    h = out_c.reshape(P, B_SH, 3, D).transpose(2, 1, 0, 3)
    scale = (scl_c.reshape(GROUPS, P)[:, PART_OF_ELEM].reshape(B_SH)
             / np.float32(126.0))
    np.multiply(h, scale[None, :, None, None], out=dst, dtype=np.float32)
    # diag_c [GROUPS, 128 p, 3*128] f32; partition p holds group element
    # E_OF_P... i.e. ordered element e sits at partition P_OF_E[e]
    dv = diag_c[:, PART_OF_ELEM, :].reshape(GROUPS, P, 3, D).transpose(
        2, 0, 1, 3).reshape(3, B_SH, D)
    dst.reshape(3, B_SH, D * D)[:, :, ::D + 1] = dv


def kernel(z1, z2):
    nc = _get_nc()
    z1 = np.ascontiguousarray(np.asarray(z1, dtype=np.float32))
    z2 = np.ascontiguousarray(np.asarray(z2, dtype=np.float32))
    in_maps = [
        {"z1": z1[c * B_SH:(c + 1) * B_SH], "z2": z2[c * B_SH:(c + 1) * B_SH]}
        for c in range(N_CORES)
    ]
    res = run_bass_kernel_spmd(nc, in_maps, core_ids=list(range(N_CORES)))
    full = np.empty((3, B_FULL, D, D), dtype=np.float32)
    for c in range(N_CORES):
        _assemble(res.results[c]["out"], res.results[c]["diag"],
                  res.results[c]["scl"], full[:, c * B_SH:(c + 1) * B_SH])
    return full


# revision 47
# speedup vs baseline: 1.1016x; 1.0553x over previous
"""Trainium2 Bass kernel for ArccosHessianCalculator (int8-packed output).

Math: for each batch element b (z1, z2 are [B, D] with D = 128):
  a = 1/|z1|, bb = 1/|z2|, c = cos = <z1u, z2u>
  Each Hessian block H_k is rank-2 plus a diagonal term:
      H_k(b) = z1 * r0_k(b)^T + z2 * r1_k(b)^T + diag
  with the normalization / cosine factors folded into r0/r1:
      k=0 (H11): r0 = -3c*a^4*z1 + a^3 b*z2          r1 = a^3 b*z1
      k=1 (H12): r0 = a^3 b*z1                        r1 = -c*a^2 b^2*z1 + a b^3*z2
      k=2 (H22): r0 = a b^3*z2                        r1 = a b^3*z1 - 3c*b^4*z2

Device strategy (per core, batch shard of 512):
  - TensorE: one K=2 matmul per element, lhsT = [z1(b); z2(b)] ([2,128] fp16),
    rhs = [r0 | r1] blocks ([2,384] fp16) -> PSUM f32 [128, 384]. Elements
    alternate PE quadrant rows ({0,1} / {32,33}) so LDWEIGHTS for element
    e+1 overlaps the stream of element e.
  - INT8 output: a guaranteed per-element bound on |H_rank2| (triangle
    inequality with |c|<=1 and AM-GM, ~16 [128,1] ops) gives s = 126/bound.
    s is folded ENTIRELY into the R coefficients, so the z fp16 casts have
    NO stats dependency (lhsT gathers fire at t=0) and PSUM holds s*H; the
    PSUM->SBUF copy is a plain int8 cast. The rank-2 part lands in DRAM
    packed as out_hw [128 i, B_SH*384 (b,k,j)] int8 -- 4x less HBM traffic
    than f32.
  - Final diagonals are computed in closed form per group ([128 b, 3*128 i])
    into a small f32 side tensor and spliced on the HOST in f32 (they
    dominate the Hessian's magnitude, so they carry no quantization error).
  - PSUM: 2-bank super-tiles x4 buffers; each drained by ONE strided copy
    (2 elements) split 5:6 between DVE and ACT (weighted by their measured
    copy rates; PSUM reads are f32 at ~1 elem/cycle/partition).
  - Stats: group 0-1 chains on DVE (short ramp); groups 2-3 and ALL diag
    (phase2) work on GPSIMD, which is otherwise idle, so recurring stats
    never steal copy throughput. Free-axis reduces / reciprocal stay on DVE
    (GPSIMD cannot do them), sqrt on ACT. All z tiles prefetch at t=0.
  - Batch rows are loaded interleaved (even elements -> partitions 0..63,
    odd -> 64..127) so gathers are plain partition slices; z and R live in
    combined tiles (zh [128,256], Rall [128,768]) so each gather half is a
    single 2-partition DMA (4 dispatches per group instead of 8).
  - Host: reshape/transpose view + per-element dequant multiply + f32
    diagonal stride-trick splice.
"""

import numpy as np
from contextlib import ExitStack

import concourse.bass as bass
import concourse.tile as tile
from concourse import bacc, mybir
from concourse.bass_utils import run_bass_kernel_spmd

N_CORES = 8
B_FULL = 4096
D = 128
B_SH = B_FULL // N_CORES  # 512 batch elements per core
P = 128                   # SBUF partitions
KD = 3 * D                # 384: three H blocks side by side
G = 32                    # elements per chunk
GROUPS = B_SH // P        # 4 stats groups of 128 elements
NCH = P // G              # 4 chunks per group
QB = 2                    # elements per PSUM super-tile (2 banks)
OC = 320                  # output cols per element (256 main + 64 packed corners)
HF = P // 2               # 64 elements per interleaved half

f32 = mybir.dt.float32
f16 = mybir.dt.float16
i8 = mybir.dt.int8

# interleaved element order within a group: partition p holds group element
# 2p (p < 64) or 2(p-64)+1 (p >= 64)
ELEM_OF_PART = np.concatenate([np.arange(0, P, 2), np.arange(1, P, 2)])
PART_OF_ELEM = np.argsort(ELEM_OF_PART)

# copy-engine schedule: 6 ACT : 5 DVE interleaved (ACT is ~1.18x faster at
# PSUM->SBUF f32 copies: (172+FD)/1.2GHz vs (120+FD)/0.96GHz).
# The first 18 super-tiles run 2:1 ACT-heavy: DVE is still draining the
# group-1 stats chain during the ramp-out.
COPY_PAT = [1, 0, 1, 0, 1, 0, 1, 0, 1, 0, 1]  # 1 = ACT


def _copy_engine(q):
    if q < 18:
        return 1 if q % 3 != 2 else 0
    return COPY_PAT[q % len(COPY_PAT)]


class _Pools:
    pass


def _make_pools(ctx, tc):
    p = _Pools()
    p.const = ctx.enter_context(tc.tile_pool(name="const", bufs=1))
    p.zg = ctx.enter_context(tc.tile_pool(name="zg", bufs=1))
    p.zh = ctx.enter_context(tc.tile_pool(name="zh", bufs=4))
    p.work = ctx.enter_context(tc.tile_pool(name="work", bufs=2))
    p.nrm = ctx.enter_context(tc.tile_pool(name="nrm", bufs=1))
    p.stat = ctx.enter_context(tc.tile_pool(name="stat", bufs=3))
    p.rall = ctx.enter_context(tc.tile_pool(name="rall", bufs=2))
    p.dpool = ctx.enter_context(tc.tile_pool(name="dpool", bufs=2))
    p.zi = ctx.enter_context(tc.tile_pool(name="zi", bufs=2))
    p.ri = ctx.enter_context(tc.tile_pool(name="ri", bufs=2))
    p.stage = ctx.enter_context(tc.tile_pool(name="stage", bufs=5))
    p.mmp = ctx.enter_context(tc.tile_pool(name="mmp", bufs=4, space="PSUM"))
    return p


def _make_consts(p, nc):
    """[128,1] constant tiles for GPSIMD (Pool has no scalar immediates)."""
    for tag, val in (("c3", 3.0), ("c075", 0.75), ("c126", 126.0),
                     ("cm1", -1.0), ("cm3", -3.0)):
        t = p.const.tile([P, 1], f32, tag=tag, name=tag)
        nc.vector.memset(t[:], val)
        setattr(p, tag, t)


def _load_all_z(p, nc, z1, z2):
    """Prefetch every group's z tiles at t=0 (tiny: 512B/partition each)."""
    zs = {}
    for grp in range(GROUPS):
        b0 = grp * P
        ldma = nc.sync if grp == 0 else nc.gpsimd
        # interleaved row order: partition p <- batch row 2p / 2(p-64)+1
        z1g = p.zg.tile([P, D], f32, tag=f"z1g{grp}", name=f"z1g_{grp}")
        ldma.dma_start(z1g[:], z1[b0:b0 + P, :].rearrange(
            "(f two) d -> two f d", two=2))
        z2g = p.zg.tile([P, D], f32, tag=f"z2g{grp}", name=f"z2g_{grp}")
        ldma.dma_start(z2g[:], z2[b0:b0 + P, :].rearrange(
            "(f two) d -> two f d", two=2))
        zs[grp] = (z1g, z2g)
    return zs


def _cast_zh(p, nc, zs, grp):
    """Plain fp16 casts of z1/z2 into ONE tile (no stats dependency)."""
    z1g, z2g = zs[grp]
    zh = p.zh.tile([P, 2 * D], f16, tag=f"zh{grp}", name=f"zh_{grp}")
    nc.scalar.copy(zh[:, 0:D], z1g[:])
    nc.scalar.copy(zh[:, D:2 * D], z2g[:])
    return zh


def _emit_zi(p, nc, zh, grp):
    """lhsT gather: one 2-partition DMA per interleaved half."""
    ZI = p.zi.tile([P, HF * D], f16, tag="ZI", name=f"ZI_{grp}")
    dmae = nc.gpsimd if grp == 1 else nc.sync
    for half in range(2):
        hb, pp = HF * half, 32 * half
        dmae.dma_start(ZI[pp:pp + 1, :], zh[hb:hb + HF, 0:D])
        dmae.dma_start(ZI[pp + 1:pp + 2, :], zh[hb:hb + HF, D:2 * D])
    return ZI


def _emit_ri(p, nc, Rall, grp):
    """rhs gather. Group 0 is split into prefix waves so chunk 0's first
    matmuls only wait on a tiny first DMA (Tile tracks write ranges), the
    rest follows while they run. Later groups: one DMA per row."""
    RI = p.ri.tile([P, HF * KD], f16, tag="RI", name=f"RI_{grp}")
    dmae = nc.gpsimd if grp == 1 else nc.sync
    waves = [(0, 12), (12, 24), (24, 44), (44, HF)] if grp == 0 else [(0, HF)]
    for lo, hi in waves:
        for half in range(2):
            hb, pp = HF * half, 32 * half
            dmae.dma_start(RI[pp:pp + 1, lo * KD:hi * KD],
                           Rall[hb + lo:hb + hi, 0:KD])
            dmae.dma_start(RI[pp + 1:pp + 2, lo * KD:hi * KD],
                           Rall[hb + lo:hb + hi, KD:2 * KD])
    return RI


def _stats_norms_tt(p, nc, zs, grp):
    """Elementwise squares/products (on the group's stats engine)."""
    gps = grp >= 2
    eng = nc.gpsimd if gps else nc.vector
    z1g, z2g = zs[grp]

    def wt(tag):
        return p.work.tile([P, D], f32, tag=tag, name=f"w_{tag}_{grp}")

    def sv(tag):
        return p.stat.tile([P, 1], f32, tag=tag, name=f"sv_{tag}_{grp}")

    def nv(tag, wide=False):
        return p.nrm.tile([P, D if wide else 1], f32, tag=f"{tag}{grp}",
                          name=f"n_{tag}_{grp}")

    st = {"eng": eng, "gps": gps, "wt": wt, "sv": sv, "nv": nv,
          "z1g": z1g, "z2g": z2g}
    v1z, v2z, wz = nv("v1z", True), nv("v2z", True), nv("wz", True)
    eng.tensor_mul(v1z[:], z1g[:], z1g[:])
    eng.tensor_mul(v2z[:], z2g[:], z2g[:])
    eng.tensor_mul(wz[:], z1g[:], z2g[:])
    st.update(v1z=v1z, v2z=v2z, wz=wz)
    return st


def _stats_norms_fin(p, nc, st, grp):
    """Reduces/reciprocals (DVE-only) + ab2t on the stats engine + ACT sqrt.
    Emitted upfront for ALL groups: the sqrt hops land on ACT before the
    copy stream starts, so they never stall copies mid-kernel."""
    eng = st["eng"]
    nv = st["nv"]
    v1z, v2z, wz = st["v1z"], st["v2z"], st["wz"]
    s1, s2, dot = nv("s1"), nv("s2"), nv("dot")
    nc.vector.reduce_sum(s1[:], v1z[:], axis=mybir.AxisListType.X)
    nc.vector.reduce_sum(s2[:], v2z[:], axis=mybir.AxisListType.X)
    nc.vector.reduce_sum(dot[:], wz[:], axis=mybir.AxisListType.X)
    mz1, mz2 = nv("mz1"), nv("mz2")
    nc.vector.reduce_max(mz1[:], st["z1g"][:], axis=mybir.AxisListType.X,
                         apply_absolute_value=True)
    nc.vector.reduce_max(mz2[:], st["z2g"][:], axis=mybir.AxisListType.X,
                         apply_absolute_value=True)
    a2, b2 = nv("a2"), nv("b2")
    nc.vector.reciprocal(a2[:], s1[:])
    nc.vector.reciprocal(b2[:], s2[:])
    ab2t, ab = nv("ab2t"), nv("ab")
    eng.tensor_mul(ab2t[:], a2[:], b2[:])
    nc.scalar.sqrt(ab[:], ab2t[:])
    st.update(s1=s1, s2=s2, dot=dot, mz1=mz1, mz2=mz2, a2=a2, b2=b2,
              ab2t=ab2t, ab=ab)
    return st


def _stats_scale(p, nc, st, grp):
    """int8 bound -> s = 126/bound, plus the s-scaled coefficient set.

    Bound (valid upper bound on max_k |H_k_rank2[i,j]|, using |c|<=1 and
    mu*mv <= (mu^2+mv^2)/2, ab <= (a2+b2)/2):
      pp = (mz1*a)^2, qq = (mz2*b)^2, pq = pp+qq
      bound = a2*(3pp+pq) + 0.75*(a2+b2)*pq + b2*(3qq+pq)
    """
    A = mybir.AluOpType
    gps = st["gps"]
    eng = st["eng"]
    sv = st["sv"]
    mz1, mz2, a2, b2, ab = st["mz1"], st["mz2"], st["a2"], st["b2"], st["ab"]
    mz1s, mz2s, pp_, qq, pq = (sv("mz1s"), sv("mz2s"), sv("pp"), sv("qq"),
                               sv("pq"))
    eng.tensor_mul(mz1s[:], mz1[:], mz1[:])
    eng.tensor_mul(mz2s[:], mz2[:], mz2[:])
    eng.tensor_mul(pp_[:], mz1s[:], a2[:])
    eng.tensor_mul(qq[:], mz2s[:], b2[:])
    eng.tensor_add(pq[:], pp_[:], qq[:])
    t11, s11, b11 = sv("t11"), sv("s11"), sv("b11")
    t22, s22, b22 = sv("t22"), sv("s22"), sv("b22")
    hh, s12, b12 = sv("hh"), sv("s12"), sv("b12")
    if gps:
        eng.tensor_mul(t11[:], pp_[:], p.c3[:])
        eng.tensor_mul(t22[:], qq[:], p.c3[:])
    else:
        eng.tensor_scalar(t11[:], pp_[:], 3.0, None, A.mult)
        eng.tensor_scalar(t22[:], qq[:], 3.0, None, A.mult)
    eng.tensor_add(s11[:], t11[:], pq[:])
    eng.tensor_mul(b11[:], s11[:], a2[:])
    eng.tensor_add(s22[:], t22[:], pq[:])
    eng.tensor_mul(b22[:], s22[:], b2[:])
    eng.tensor_add(hh[:], a2[:], b2[:])
    eng.tensor_mul(s12[:], pq[:], hh[:])
    if gps:
        eng.tensor_mul(b12[:], s12[:], p.c075[:])
    else:
        eng.tensor_scalar(b12[:], s12[:], 0.75, None, A.mult)
    bs, inv, s = sv("bs"), sv("inv"), sv("s")
    bound = st["nv"]("bound")
    eng.tensor_add(bs[:], b11[:], b12[:])
    eng.tensor_add(bound[:], bs[:], b22[:])
    st["bound"] = bound
    return st


def _stats_coef(p, nc, st, grp):
    """reciprocal (DVE) + s + the s-scaled coefficient chain. For GPSIMD
    groups this is emitted in-loop AFTER the bound chain has long completed,
    so the lone DVE reciprocal never stalls the copy queue."""
    A = mybir.AluOpType
    gps = st["gps"]
    eng = st["eng"]
    sv = st["sv"]
    a2, b2, ab, bound = st["a2"], st["b2"], st["ab"], st["bound"]
    inv, s = sv("inv"), sv("s")
    nc.vector.reciprocal(inv[:], bound[:])
    if gps:
        eng.tensor_mul(s[:], inv[:], p.c126[:])
    else:
        eng.tensor_scalar(s[:], inv[:], 126.0, None, A.mult)

    # ---- coefficient chain (c, diag helpers, s-scaled R coefficients) ----
    c, m3c, mc, mcab = sv("c"), sv("m3c"), sv("mc"), sv("mcab")
    eng.tensor_mul(c[:], st["dot"][:], ab[:])
    if gps:
        eng.tensor_mul(m3c[:], c[:], p.cm3[:])
        eng.tensor_mul(mc[:], c[:], p.cm1[:])
    else:
        eng.tensor_scalar(m3c[:], c[:], -3.0, None, A.mult)
        eng.tensor_scalar(mc[:], c[:], -1.0, None, A.mult)
    eng.tensor_mul(mcab[:], mc[:], ab[:])
    A3B, AB3, A4, B4, A2B2 = (sv("A3B"), sv("AB3"), sv("A4"), sv("B4"),
                              sv("A2B2"))
    eng.tensor_mul(A3B[:], a2[:], ab[:])
    eng.tensor_mul(AB3[:], b2[:], ab[:])
    eng.tensor_mul(A4[:], a2[:], a2[:])
    eng.tensor_mul(B4[:], b2[:], b2[:])
    eng.tensor_mul(A2B2[:], ab[:], ab[:])
    cs, m3cs, mcs = sv("cs"), sv("m3cs"), sv("mcs")
    eng.tensor_mul(cs[:], c[:], s[:])
    if gps:
        eng.tensor_mul(m3cs[:], cs[:], p.cm3[:])
        eng.tensor_mul(mcs[:], cs[:], p.cm1[:])
    else:
        eng.tensor_scalar(m3cs[:], cs[:], -3.0, None, A.mult)
        eng.tensor_scalar(mcs[:], cs[:], -1.0, None, A.mult)
    alv, bev, gav, dev, epv = (sv("alv"), sv("bev"), sv("gav"), sv("dev"),
                               sv("epv"))
    eng.tensor_mul(alv[:], m3cs[:], A4[:])
    eng.tensor_mul(bev[:], A3B[:], s[:])
    eng.tensor_mul(gav[:], mcs[:], A2B2[:])
    eng.tensor_mul(dev[:], AB3[:], s[:])
    eng.tensor_mul(epv[:], m3cs[:], B4[:])
    st.update(c=c, m3c=m3c, mcab=mcab,
              alv=alv, bev=bev, gav=gav, dev=dev, epv=epv)
    return st


def _stats_phase1(p, nc, st, grp):
    """R rows in fp16, s pre-folded, packed in ONE tile:
    Rall = [R0 | R1] = [A | B | Dd  |  B | C | E] with
      A = alv*z1 + bev*z2   B = bev*z1   Dd = dev*z2
      C = gav*z1 + dev*z2   E = dev*z1 + epv*z2
    """
    A = mybir.AluOpType
    gps = st["gps"]
    eng = st["eng"]
    z1g, z2g, wt = st["z1g"], st["z2g"], st["wt"]
    Rall = p.rall.tile([P, 2 * KD], f16, tag="Rall", name=f"Rall_{grp}")

    def ts(out, in0, svt):
        if gps:
            eng.tensor_mul(out, in0, svt[:].broadcast_to([P, in0.shape[-1]]))
        else:
            eng.tensor_scalar(out, in0, svt[:], None, A.mult)

    def stt(out, in0, svt, in1, tag):
        if gps:
            w = in0.shape[-1]
            tmp = wt(tag)
            eng.tensor_mul(tmp[:, 0:w], in0,
                           svt[:].broadcast_to([P, w]))
            eng.tensor_add(out, tmp[:, 0:w], in1)
        else:
            eng.scalar_tensor_tensor(out, in0, svt[:], in1, A.mult, A.add)

    alv, bev, gav, dev, epv = (st["alv"], st["bev"], st["gav"], st["dev"],
                               st["epv"])
    t0 = wt("t0")
    ts(t0[:], z2g[:], bev)
    stt(Rall[:, 0:D], z1g[:], alv, t0[:], "p1a")           # A
    ts(Rall[:, D:2 * D], z1g[:], bev)                      # B
    ts(Rall[:, 2 * D:3 * D], z2g[:], dev)                  # Dd
    ts(Rall[:, 3 * D:4 * D], z1g[:], bev)                  # B (R1 copy)
    t1 = wt("t1")
    ts(t1[:], z2g[:], dev)
    stt(Rall[:, 4 * D:5 * D], z1g[:], gav, t1[:], "p1b")   # C
    t2 = wt("t2")
    ts(t2[:], z2g[:], epv)
    stt(Rall[:, 5 * D:6 * D], z1g[:], dev, t2[:], "p1c")   # E
    st["Rall"] = Rall


def _stats_phase2(p, nc, st, grp, diag_hw):
    """Final diagonal values, batch-major [128b, 3*128i]; DMA'd out as f32.
    ALWAYS on GPSIMD (latency-irrelevant; keeps DVE/ACT for copies).
    Host overwrites out[k, b, i, i] with these.
    """
    eng = nc.gpsimd
    wt = st["wt"]

    def sv(tag):
        return p.stat.tile([P, 1], f32, tag=tag, name=f"sv_{tag}_{grp}")

    v1z, v2z, wz = st["v1z"], st["v2z"], st["wz"]
    a2, b2, ab, c, m3c = st["a2"], st["b2"], st["ab"], st["c"], st["m3c"]
    dall = p.dpool.tile([P, KD], f32, tag="dall", name=f"dall_{grp}")

    def bc(svt):
        return svt[:].broadcast_to([P, D])

    twoabw = wt("twoabw")
    ab2 = sv("ab2")
    eng.tensor_add(ab2[:], ab[:], ab[:])
    eng.tensor_mul(twoabw[:], wz[:], bc(ab2))
    # d11 = a2*(c + 2ab*wz + m3c*a2*v1z)
    u1, u2, u2c = wt("u1"), wt("u2"), wt("u2c")
    pa = sv("pa")
    eng.tensor_mul(pa[:], a2[:], m3c[:])
    eng.tensor_mul(u1[:], v1z[:], bc(pa))
    eng.tensor_add(u2[:], u1[:], twoabw[:])
    eng.tensor_add(u2c[:], u2[:], bc(c))
    eng.tensor_mul(dall[:, 0:D], u2c[:], bc(a2))
    # d12 = ab*(a2*v1z + b2*v2z + mcab*wz - 1)
    w1, w2, w2b, w3, w3b, w4 = (wt("w1"), wt("w2"), wt("w2b"), wt("w3"),
                                wt("w3b"), wt("w4"))
    eng.tensor_mul(w1[:], v1z[:], bc(a2))
    eng.tensor_mul(w2[:], v2z[:], bc(b2))
    eng.tensor_add(w2b[:], w2[:], w1[:])
    eng.tensor_mul(w3[:], wz[:], bc(st["mcab"]))
    eng.tensor_add(w3b[:], w3[:], w2b[:])
    eng.tensor_mul(w4[:], w3b[:], bc(ab))
    eng.tensor_sub(dall[:, D:2 * D], w4[:], bc(ab))
    # d22 = b2*(c + 2ab*wz + m3c*b2*v2z)
    u3, u4, u4c = wt("u3"), wt("u4"), wt("u4c")
    pb = sv("pb")
    eng.tensor_mul(pb[:], b2[:], m3c[:])
    eng.tensor_mul(u3[:], v2z[:], bc(pb))
    eng.tensor_add(u4[:], u3[:], twoabw[:])
    eng.tensor_add(u4c[:], u4[:], bc(c))
    eng.tensor_mul(dall[:, 2 * D:3 * D], u4c[:], bc(b2))
    nc.gpsimd.dma_start(diag_hw[grp], dall[:])
    # dequant scale (bound) to DRAM; deferred here so it never blocks the
    # GPSIMD queue head during the ramp
    nc.gpsimd.dma_start(p.scl_hw[grp], st["bound"][:])


def _emit_chunk(p, nc, ZI, RI, out_hw, grp, ch, qctr):
    """G elements (matmul + quad PSUM->int8 copy) + output DMA."""
    e0 = grp * P + ch * G     # global element base for this chunk
    ci = grp * NCH + ch
    GS = 8 if ci in (0, GROUPS * NCH - 1) else 16
    for sub in range(G // GS):
        STG = p.stage.tile([P, GS * KD], i8, tag="STG",
                           name=f"STG_{grp}_{ch}_{sub}")
        for t in range(GS // QB):
            pt = p.mmp.tile([P, QB * 512], f32, tag="pt",
                            name=f"pt_{grp}_{ch}_{sub}_{t}")
            for slot in range(QB):
                s = sub * GS + t * QB + slot   # local element in chunk
                half = s % 2                   # PE quadrant alternation
                ff = ch * (G // 2) + s // 2    # column within group tile
                pp = 32 * half
                lhsT = ZI[pp:pp + 2, ff * D:(ff + 1) * D]
                rhs = RI[pp:pp + 2, ff * KD:(ff + 1) * KD]
                nc.tensor.matmul(pt[:, slot * 512:slot * 512 + KD],
                                 lhsT, rhs, start=True, stop=True)
            src = pt[:].rearrange("p (q c) -> p q c", c=512)[:, :, 0:KD]
            dst = STG[:, t * QB * KD:(t + 1) * QB * KD].rearrange(
                "p (q c) -> p q c", c=KD)
            if _copy_engine(qctr[0]):
                nc.scalar.copy(dst, src)
            else:
                nc.vector.tensor_copy(dst, src)
            qctr[0] += 1
        es = e0 + sub * GS
        nc.sync.dma_start(out_hw[:, es * KD:(es + GS) * KD], STG[:])


def _build_body(ctx, tc, z1, z2, out_hw, diag_hw, scl_hw):
    nc = tc.nc
    p = _make_pools(ctx, tc)
    p.scl_hw = scl_hw
    _make_consts(p, nc)

    qctr = [0]
    zs = _load_all_z(p, nc, z1, z2)
    # z fp16 casts + lhsT gathers have NO stats dependency -> fire at t=0
    zh = {0: _cast_zh(p, nc, zs, 0), 1: _cast_zh(p, nc, zs, 1)}
    gzi = {0: _emit_zi(p, nc, zh[0], 0), 1: _emit_zi(p, nc, zh[1], 1)}
    # group 0-1 stats on DVE during the ramp window
    # group 0 chain on DVE: the ramp-critical path
    sts = {0: _stats_norms_tt(p, nc, zs, 0)}
    _stats_norms_fin(p, nc, sts[0], 0)
    _stats_scale(p, nc, sts[0], 0)
    _stats_coef(p, nc, sts[0], 0)
    _stats_phase1(p, nc, sts[0], 0)
    gri = {0: _emit_ri(p, nc, sts[0]["Rall"], 0)}
    # groups 2-3 norms upfront (GPSIMD TTs + DVE reduces + ACT sqrts): their
    # sqrt hops run on ACT BEFORE the copy stream starts, never stalling it
    sts[2] = _stats_norms_tt(p, nc, zs, 2)
    sts[3] = _stats_norms_tt(p, nc, zs, 3)
    _stats_norms_fin(p, nc, sts[2], 2)
    _stats_norms_fin(p, nc, sts[3], 3)
    # group 1 chain on DVE
    sts[1] = _stats_norms_tt(p, nc, zs, 1)
    _stats_norms_fin(p, nc, sts[1], 1)
    _stats_scale(p, nc, sts[1], 1)
    _stats_coef(p, nc, sts[1], 1)
    _stats_phase1(p, nc, sts[1], 1)
    # groups 2-3 bound chains: pure GPSIMD, all inputs ready -> run in the
    # ramp window, emitted BEFORE RI1 whose Rall1-wait blocks the GPSIMD
    # queue head until ~32us
    _stats_scale(p, nc, sts[2], 2)
    _stats_scale(p, nc, sts[3], 3)
    gri[1] = _emit_ri(p, nc, sts[1]["Rall"], 1)
    # diag work entirely on GPSIMD, never latency-critical
    _stats_phase2(p, nc, sts[0], 0, diag_hw)
    _stats_phase2(p, nc, sts[1], 1, diag_hw)
    zh[2] = _cast_zh(p, nc, zs, 2)
    zh[3] = _cast_zh(p, nc, zs, 3)
    TOT = GROUPS * NCH
    for ci in range(TOT):
        grp, ch = divmod(ci, NCH)
        if ch == 0 and grp in (1, 2):
            # groups 2-3: gathers 3.5 chunks ahead of first use, emitted as
            # soon as the previous group's chunks (the buffer's last readers)
            # are all emitted, so the GPSIMD queue has slack to deliver them
            gzi[grp + 1] = _emit_zi(p, nc, zh[grp + 1], grp + 1)
            gri[grp + 1] = _emit_ri(p, nc, sts[grp + 1]["Rall"], grp + 1)
        _emit_chunk(p, nc, gzi[grp], gri[grp], out_hw, grp, ch, qctr)
        # remaining stats for group g+2 on GPSIMD (norms already done in the
        # ramp window; only the GPS-local coef/R/diag chains remain)
        if grp + 2 < GROUPS:
            if ch == 1:
                _stats_coef(p, nc, sts[grp + 2], grp + 2)
            elif ch == 2:
                _stats_phase1(p, nc, sts[grp + 2], grp + 2)
            elif ch == 3:
                _stats_phase2(p, nc, sts[grp + 2], grp + 2, diag_hw)


def build_kernel():
    nc = bacc.Bacc("TRN2", target_bir_lowering=False, debug=False)
    z1 = nc.dram_tensor("z1", [B_SH, D], f32, kind="ExternalInput").ap()
    z2 = nc.dram_tensor("z2", [B_SH, D], f32, kind="ExternalInput").ap()
    # packed rank-2 output: [i partition, (b, k, j) free], int8
    out_hw = nc.dram_tensor("out", [P, B_SH * KD], i8,
                            kind="ExternalOutput").ap()
    scl_hw = nc.dram_tensor("scl", [GROUPS, P, 1], f32,
                            kind="ExternalOutput").ap()
    # final diagonals: [group, b-partition (interleaved), (k, i) free], f32
    diag_hw = nc.dram_tensor("diag", [GROUPS, P, KD], f32,
                             kind="ExternalOutput").ap()
    with tile.TileContext(nc) as tc:
        with ExitStack() as ctx:
            _build_body(ctx, tc, z1, z2, out_hw, diag_hw, scl_hw)
    nc.compile()
    return nc


_NC_CACHE = None


def _get_nc():
    global _NC_CACHE
    if _NC_CACHE is None:
        _NC_CACHE = build_kernel()
    return _NC_CACHE


def _assemble(out_c, diag_c, scl_c, dst):
    """Unpack one core's HW tensors into dst [3, B_SH, D, D] f32."""
    # out_c [128 i, B_SH*384] int8 -> [i, b, k, j] -> [k, b, i, j]; dequant
    # with the per-element scale bound/126 (scl rows are partition-ordered)
    h = out_c.reshape(P, B_SH, 3, D).transpose(2, 1, 0, 3)
    scale = (scl_c.reshape(GROUPS, P)[:, PART_OF_ELEM].reshape(B_SH)
             / np.float32(126.0))
    np.multiply(h, scale[None, :, None, None], out=dst, dtype=np.float32)
    # diag_c [GROUPS, 128 p, 3*128] f32; partition p holds group element
    # E_OF_P... i.e. ordered element e sits at partition P_OF_E[e]
    dv = diag_c[:, PART_OF_ELEM, :].reshape(GROUPS, P, 3, D).transpose(
        2, 0, 1, 3).reshape(3, B_SH, D)
    dst.reshape(3, B_SH, D * D)[:, :, ::D + 1] = dv


def kernel(z1, z2):
    nc = _get_nc()
    z1 = np.ascontiguousarray(np.asarray(z1, dtype=np.float32))
    z2 = np.ascontiguousarray(np.asarray(z2, dtype=np.float32))
    in_maps = [
        {"z1": z1[c * B_SH:(c + 1) * B_SH], "z2": z2[c * B_SH:(c + 1) * B_SH]}
        for c in range(N_CORES)
    ]
    res = run_bass_kernel_spmd(nc, in_maps, core_ids=list(range(N_CORES)))
    full = np.empty((3, B_FULL, D, D), dtype=np.float32)
    for c in range(N_CORES):
        _assemble(res.results[c]["out"], res.results[c]["diag"],
                  res.results[c]["scl"], full[:, c * B_SH:(c + 1) * B_SH])
    return full
